# revision 1
# baseline (speedup 1.0000x reference)
"""Trainium2 Bass kernel for an AttentionBlock (GroupNorm + single-layer MHA + proj residual).

Reference computation (per batch b):
    xn = GroupNorm(x[b])                        # 8 groups over C=256, HW spatial
    qkv = w_qkv @ xn                            # per-pixel 1x1 conv
    per head h (4 heads, d=64):
        scores = q_h^T k_h * d^-0.5             # [HW, HW]
        attn = softmax(scores, axis=keys)
        out_h = v_h @ attn^T                    # [d, HW]
    y = xn + w_proj @ concat(out_h) + b_proj

Sharding: 8 cores = (batch b in {0,1}) x (query quarter s in {0..3}).  Each
core runs GroupNorm, computes k/v for ALL spatial positions and q for its
own quarter, then runs all 4 heads' attention for its own 1024 query
columns.  The head sum of the projection is a local PSUM accumulation, so
there is NO collective at all: each core writes its own [C, 1024] slice of
the output, with the residual fused into the PSUM drain.

Key kernel-level layout choices:
 - scores are computed TRANSPOSED (keys j on partitions, queries i on the
   free axis) so the PV contraction (over j) needs no transposes, and the
   softmax denominator comes free as a 65th "ones" column of V.
 - softmax skips max-subtraction (scores ~N(0,1)); scores live in the log2
   domain (q pre-scaled by d^-0.5*log2 e host-side).
 - exp work (the dominant engine cost: 256 [128,512] tiles/core) is SPLIT
   between the Scalar engine (native EXP) and the Vector engine, which
   computes 2^t with one tensor_scalar per tile: int16(t*128 + 16256.5) is
   bit-identical to bf16(2^t) up to the standard linear-mantissa approx
   (~2% noise that cancels through the softmax normalization).
 - the score/exp/PV chain is software-pipelined with a 3-half lookahead so
   exp latency stays off the PE critical path; two K=64 score matmuls are
   row-packed per PE pass (k/q duplicated into both partition halves by
   SBUF-to-SBUF DMA, off the compute engines).
 - attention matmuls run in bf16 with fp32 PSUM accumulation.
"""

import numpy as np

C = 256
NH = 4
D = 64
G = 8
EPS = 1e-5
B = 2
NCORES = 8
PDIM = 128  # partitions


def build_nc(HW: int):
    import concourse.bass as bass
    import concourse.mybir as mybir
    import concourse.tile as tile
    from concourse import bacc

    f32 = mybir.dt.float32
    bf16 = mybir.dt.bfloat16
    i16 = mybir.dt.int16
    CW = min(512, HW)          # i-chunk width (matmul moving-operand max)
    NIC = HW // CW             # number of column chunks of the full image
    OWN = HW // 4              # query columns owned per core
    NOC = OWN // CW            # own-column chunks
    NJT = HW // PDIM           # number of key tiles (128 keys each)
    LA = 2                     # pv lookahead in key-tile halves

    nc = bacc.Bacc(
        "TRN2", target_bir_lowering=False, debug=False, num_devices=NCORES
    )

    xb = nc.declare_dram_parameter("xb", [C, HW], f32, isOutput=False)
    x_own = nc.declare_dram_parameter("x_own", [C, OWN], f32, isOutput=False)
    wqT = nc.declare_dram_parameter("wqT", [C, C], bf16, isOutput=False)
    wkT = nc.declare_dram_parameter("wkT", [C, C], bf16, isOutput=False)
    wvT = nc.declare_dram_parameter("wvT", [C, C], bf16, isOutput=False)
    wpT = nc.declare_dram_parameter("wpT", [C, C], bf16, isOutput=False)
    gamma = nc.declare_dram_parameter("gamma", [C], f32, isOutput=False)
    beta = nc.declare_dram_parameter("beta", [C], f32, isOutput=False)
    bproj = nc.declare_dram_parameter("bproj", [C], f32, isOutput=False)
    indf = nc.declare_dram_parameter("indf", [2, PDIM, G], f32, isOutput=False)
    indb = nc.declare_dram_parameter("indb", [2, G, PDIM], f32, isOutput=False)
    y = nc.declare_dram_parameter("y", [C, OWN], f32, isOutput=True)


    Exp = mybir.ActivationFunctionType.Exp
    Sqrt = mybir.ActivationFunctionType.Sqrt
    MUL = mybir.AluOpType.mult
    ADD = mybir.AluOpType.add

    BNW = min(512, HW)         # bn_stats max free dim
    NBN = HW // BNW

    with tile.TileContext(nc) as tc:
        with (
            tc.tile_pool(name="consts", bufs=1) as consts,
            tc.tile_pool(name="xpool", bufs=1) as xpool,
            tc.tile_pool(name="xnpool", bufs=1) as xnpool,
            tc.tile_pool(name="gn_sm", bufs=2) as gn_sm,
            tc.tile_pool(name="qkpool", bufs=1) as qkpool,
            tc.tile_pool(name="espool", bufs=4) as espool,
            tc.tile_pool(name="mlsm", bufs=3) as mlsm,
            tc.tile_pool(name="ypool", bufs=4) as ypool,
        ):
            # ---------------- x load first (biggest transfer, gates GN) ----------------
            # Spread the 4MB x load over the three DMA-capable engine queues.
            dma_engines = [nc.sync, nc.scalar, nc.gpsimd]
            x_sb = []
            xo_sb = []
            di = 0
            for t in range(2):
                xt = xpool.tile([PDIM, HW], f32, tag=f"x{t}", name=f"x{t}")
                for c in range(NIC):
                    dma_engines[di % 3].dma_start(
                        out=xt[:, bass.ts(c, CW)],
                        in_=xb[bass.ts(t, PDIM), bass.ts(c, CW)],
                    )
                    di += 1
                x_sb.append(xt)
                xo = xpool.tile([PDIM, OWN], f32, tag=f"xo{t}", name=f"xo{t}")
                nc.sync.dma_start(out=xo, in_=x_own[bass.ts(t, PDIM), :])
                xo_sb.append(xo)

            # ---------------- constants / small loads ----------------
            eps_t = consts.tile([PDIM, 1], f32)
            nc.vector.memset(eps_t, EPS)
            ones64 = consts.tile([1, D], bf16)
            nc.vector.memset(ones64, 1.0)

            indf_sb = []
            indb_sb = []
            gm_sb = []
            bt_sb = []
            bp_sb = []
            for t in range(2):
                it_ = consts.tile([PDIM, G], f32, tag=f"indf{t}")
                nc.sync.dma_start(out=it_, in_=indf[t])
                indf_sb.append(it_)
                ib_ = consts.tile([G, PDIM], f32, tag=f"indb{t}")
                nc.sync.dma_start(out=ib_, in_=indb[t])
                indb_sb.append(ib_)
                g_ = consts.tile([PDIM, 1], f32, tag=f"gm{t}")
                nc.sync.dma_start(out=g_, in_=gamma[bass.ts(t, PDIM)].rearrange("(p o) -> p o", o=1))
                gm_sb.append(g_)
                b_ = consts.tile([PDIM, 1], f32, tag=f"bt{t}")
                nc.sync.dma_start(out=b_, in_=beta[bass.ts(t, PDIM)].rearrange("(p o) -> p o", o=1))
                bt_sb.append(b_)
                bp_ = consts.tile([PDIM, 1], f32, tag=f"bp{t}")
                nc.sync.dma_start(out=bp_, in_=bproj[bass.ts(t, PDIM)].rearrange("(p o) -> p o", o=1))
                bp_sb.append(bp_)

            # weight tiles: [c-half t, 256 outputs] each; wp per head
            wq_sb, wk_sb, wv_sb = [], [], []
            for t in range(2):
                for (w_sb, src, tag) in (
                    (wq_sb, wqT, "wq"),
                    (wk_sb, wkT, "wk"),
                    (wv_sb, wvT, "wv"),
                ):
                    wt = consts.tile([PDIM, C], bf16, tag=f"{tag}{t}")
                    nc.sync.dma_start(out=wt, in_=src[bass.ts(t, PDIM), :])
                    w_sb.append(wt)
            wp_sb = []
            for h in range(NH):
                wt = consts.tile([D, C], bf16, tag=f"wp{h}", name=f"wp{h}")
                nc.sync.dma_start(out=wt, in_=wpT[h * D : (h + 1) * D, :])
                wp_sb.append(wt)

            from contextlib import ExitStack

            ps_stack = ExitStack()
            gn_ps = ps_stack.enter_context(tc.tile_pool(name="gn_ps", bufs=1, space="PSUM"))
            qk_ps = ps_stack.enter_context(tc.tile_pool(name="qk_ps", bufs=3, space="PSUM"))

            gst_ps = gn_ps.tile([G, 2], f32, tag="gst")
            for t in range(2):
                stats = gn_sm.tile([PDIM, NBN, 6], f32, tag="bnst")
                for s in range(NBN):
                    nc.vector.bn_stats(out=stats[:, s, :], in_=x_sb[t][:, bass.ts(s, BNW)])
                mv = gn_sm.tile([PDIM, 2], f32, tag="mv")
                nc.vector.bn_aggr(out=mv, in_=stats)
                st2 = gn_sm.tile([PDIM, 2], f32, tag="st2")
                nc.vector.tensor_copy(st2[:, 0:1], mv[:, 0:1])
                sq = gn_sm.tile([PDIM, 1], f32, tag="sq")
                nc.vector.tensor_mul(sq, mv[:, 0:1], mv[:, 0:1])
                nc.vector.tensor_add(st2[:, 1:2], mv[:, 1:2], sq)
                nc.tensor.matmul(
                    out=gst_ps, lhsT=indf_sb[t], rhs=st2, start=(t == 0), stop=(t == 1)
                )

            gst = gn_sm.tile([G, 2], f32, tag="gst_sb")
            nc.vector.tensor_copy(gst, gst_ps)
            mu2 = gn_sm.tile([G, 1], f32, tag="mu2")
            nc.vector.tensor_mul(mu2, gst[:, 0:1], gst[:, 0:1])
            var = gn_sm.tile([G, 1], f32, tag="var")
            nc.vector.tensor_sub(var, gst[:, 1:2], mu2)
            sd = gn_sm.tile([G, 1], f32, tag="sd")
            nc.scalar.activation(out=sd, in_=var, func=Sqrt, bias=eps_t[0:G, :], scale=1.0)
            rstd = gn_sm.tile([G, 1], f32, tag="rstd")
            nc.vector.reciprocal(out=rstd, in_=sd)
            gmr = gn_sm.tile([G, 2], f32, tag="gmr")
            nc.vector.tensor_copy(gmr[:, 0:1], gst[:, 0:1])
            nc.vector.tensor_copy(gmr[:, 1:2], rstd)

            # per-channel affine params + normalized x + residual slice
            xn_sb = []
            resid_sb = []
            xn_own_sb = []
            for t in range(2):
                gb_ps = gn_ps.tile([PDIM, 2], f32, tag="gb")
                nc.tensor.matmul(out=gb_ps, lhsT=indb_sb[t], rhs=gmr, start=True, stop=True)
                gb = gn_sm.tile([PDIM, 2], f32, tag="gb_sb")
                nc.vector.tensor_copy(gb, gb_ps)
                A_t = gn_sm.tile([PDIM, 1], f32, tag=f"A{t}")
                nc.vector.tensor_mul(A_t, gb[:, 1:2], gm_sb[t])
                tmp = gn_sm.tile([PDIM, 1], f32, tag="tmp")
                nc.vector.tensor_mul(tmp, gb[:, 0:1], A_t)
                B_t = gn_sm.tile([PDIM, 1], f32, tag=f"B{t}")
                nc.vector.tensor_sub(B_t, bt_sb[t], tmp)
                B2_t = gn_sm.tile([PDIM, 1], f32, tag=f"B2{t}")
                nc.vector.tensor_add(B2_t, B_t, bp_sb[t])

                xn_t = xnpool.tile([PDIM, HW], bf16, tag=f"xn{t}")
                # per-chunk ops split DVE/gpsimd: subtile deps let the first
                # qkv matmuls start as soon as their xn slice exists
                for cc in range(NIC):
                    eng = nc.vector if (cc % 4 != 3) else nc.gpsimd
                    eng.tensor_scalar(
                        xn_t[:, bass.ts(cc, CW)],
                        x_sb[t][:, bass.ts(cc, CW)],
                        A_t, B_t, MUL, ADD,
                    )
                xn_sb.append(xn_t)
                rs_t = xnpool.tile([PDIM, OWN], f32, tag=f"res{t}")
                nc.gpsimd.tensor_scalar(rs_t, xo_sb[t], A_t, B2_t, MUL, ADD)
                resid_sb.append(rs_t)
                xno_t = xnpool.tile([PDIM, OWN], bf16, tag=f"xno{t}")
                nc.gpsimd.tensor_scalar(xno_t, xo_sb[t], A_t, B_t, MUL, ADD)
                xn_own_sb.append(xno_t)

            # ---------------- k, q, v production (undup) ----------------
            # k/q: [256 out-ch = 4 heads x 64, cols]; heads 0,1 in out-half 0.
            ku = [qkpool.tile([PDIM, HW], bf16, tag=f"ku{co}", name=f"ku{co}") for co in range(2)]
            for co in range(2):
                for c in range(NIC):
                    ps = qk_ps.tile([PDIM, CW], f32, tag="qk")
                    for t in range(2):
                        nc.tensor.matmul(
                            out=ps,
                            lhsT=wk_sb[t][:, bass.ts(co, PDIM)],
                            rhs=xn_sb[t][:, bass.ts(c, CW)],
                            start=(t == 0),
                            stop=(t == 1),
                        )
                    eng = nc.scalar if (c % 2 == 0) else nc.vector
                    if eng is nc.scalar:
                        eng.copy(ku[co][:, bass.ts(c, CW)], ps)
                    else:
                        eng.tensor_copy(ku[co][:, bass.ts(c, CW)], ps)

            qu = [qkpool.tile([PDIM, OWN], bf16, tag=f"qu{co}", name=f"qu{co}") for co in range(2)]
            for co in range(2):
                for c in range(NOC):
                    ps = qk_ps.tile([PDIM, CW], f32, tag="qk")
                    for t in range(2):
                        nc.tensor.matmul(
                            out=ps,
                            lhsT=wq_sb[t][:, bass.ts(co, PDIM)],
                            rhs=xn_own_sb[t][:, bass.ts(c, CW)],
                            start=(t == 0),
                            stop=(t == 1),
                        )
                    nc.scalar.copy(qu[co][:, bass.ts(c, CW)], ps)

            # v: per key tile, all 4 heads at once -> strided into v4 slots.
            # Only the first 8 key tiles are produced up front; the rest are
            # emitted inside the first vchunk's pair loop so their PSUM
            # drains overlap the attention pipeline instead of the prologue.
            v4 = qkpool.tile([PDIM, NJT, NH, D + 1], bf16, tag="v4")
            nc.vector.memset(v4[:, :, :, D : D + 1], 1.0)

            def emit_v(jt, pool=None):
                ps = (pool or qk_ps).tile([PDIM, C], f32, tag="vt", name="vt")
                for t in range(2):
                    nc.tensor.matmul(
                        out=ps,
                        lhsT=xn_sb[t][:, bass.ts(jt, PDIM)],
                        rhs=wv_sb[t],
                        start=(t == 0),
                        stop=(t == 1),
                    )
                eng = nc.scalar if (jt % 2 == 0) else nc.vector
                if eng is nc.scalar:
                    eng.copy(v4[:, jt, :, 0:D], ps.rearrange("p (h d) -> p h d", h=NH))
                else:
                    eng.tensor_copy(v4[:, jt, :, 0:D], ps.rearrange("p (h d) -> p h d", h=NH))

            for jt in range(NJT):
                emit_v(jt)

            # ---------------- swapped-halves companion tiles ----------------
            # Score pair packing needs each head's k/q in BOTH partition
            # halves.  ku/qu already hold head 2cp in the lower half and
            # head 2cp+1 in the upper; ONE extra tile per pair with the
            # halves swapped covers the other slot of each head, halving
            # the SBUF-to-SBUF duplication traffic (DMA, off the engines).
            kx = [qkpool.tile([PDIM, HW], bf16, tag=f"kx{cp}", name=f"kx{cp}") for cp in range(2)]
            qx = [qkpool.tile([PDIM, OWN], bf16, tag=f"qx{cp}", name=f"qx{cp}") for cp in range(2)]
            for cp in range(2):
                nc.sync.dma_start(out=kx[cp][0:D, :], in_=ku[cp][D : 2 * D, :])
                nc.sync.dma_start(out=kx[cp][D : 2 * D, :], in_=ku[cp][0:D, :])
                nc.scalar.dma_start(out=qx[cp][0:D, :], in_=qu[cp][D : 2 * D, :])
                nc.scalar.dma_start(out=qx[cp][D : 2 * D, :], in_=qu[cp][0:D, :])

            def k_src(h, s):
                # head h's k at partition half s
                return (ku if (h % 2) == s else kx)[h // 2]

            def q_src(h, s):
                return (qu if (h % 2) == s else qx)[h // 2]


            # ---------------- main attention loop ----------------
            ps_stack.close()  # release GN/QKV PSUM banks
            ps_stack2 = ExitStack()
            sc_ps = ps_stack2.enter_context(tc.tile_pool(name="sc_ps", bufs=4, space="PSUM"))
            pv_ps_pool = ps_stack2.enter_context(tc.tile_pool(name="pv_ps", bufs=2, space="PSUM"))
            pj_ps_pool = ps_stack2.enter_context(tc.tile_pool(name="pj_ps", bufs=1, space="PSUM"))

            # Per (i-chunk, head) "vchunk": 32 score/exp/PV halves, pipelined
            # with LA lookahead.  Each vchunk's normalization chain (den ->
            # recip -> broadcast -> onorm) is DEFERRED into the next vchunk's
            # half-loop so it fills pipeline slack instead of serializing at
            # the boundary; the projection (4-head PSUM accumulation + fused
            # residual) emits once its chunk's 4 onorms exist.
            onorms_by_cc = [[] for _ in range(NOC)]

            def emit_proj(cc):
                cslice = bass.ts(cc, CW)
                for co in range(2):
                    pj = pj_ps_pool.tile([PDIM, CW], f32, tag=f"pj{co}", bufs=1, name=f"pj{co}")
                    for h in range(NH):
                        nc.tensor.matmul(
                            out=pj,
                            lhsT=wp_sb[h][:, bass.ts(co, PDIM)],
                            rhs=onorms_by_cc[cc][h],
                            start=(h == 0),
                            stop=(h == NH - 1),
                        )
                    yf = ypool.tile([PDIM, CW], f32, tag="yf", name="yf")
                    nc.vector.tensor_add(yf, pj, resid_sb[co][:, cslice])
                    nc.sync.dma_start(out=y[bass.ts(co, PDIM), cslice], in_=yf)

            def make_chain(cc, pv):
                state = {}

                def stage1():
                    pvs = mlsm.tile([D, CW], f32, tag="pvs", name="pvs")
                    nc.scalar.copy(pvs, pv[0:D, :])
                    den = mlsm.tile([1, CW], f32, tag="den", name="den")
                    nc.vector.tensor_copy(den, pv[D : D + 1, :])
                    rden = mlsm.tile([1, CW], f32, tag="rden", name="rden")
                    nc.vector.reciprocal_approx_fast(out=rden, in_=den)
                    rdenb = mlsm.tile([1, CW], f32, tag="rdenb", name="rdenb")
                    nc.gpsimd.tensor_copy(rdenb, rden)
                    rdb = mlsm.tile([D, CW], f32, tag="rdb", name="rdb", bufs=2)
                    nc.gpsimd.partition_broadcast(rdb, rdenb[:, :])
                    state["pvs"], state["rdb"] = pvs, rdb

                def stage2():
                    onorm = mlsm.tile([D, CW], bf16, tag="onorm", bufs=5, name="onorm")
                    nc.vector.tensor_mul(onorm, state["rdb"], state["pvs"])
                    onorms_by_cc[cc].append(onorm)
                    if len(onorms_by_cc[cc]) == NH:
                        emit_proj(cc)
                return stage1, stage2

            deferred = None
            # One flat pair-stream over (chunk, head): the pend queue carries
            # ACROSS vchunk boundaries, so the final PV pops of one head
            # interleave with the next head's first score/exp pairs and the
            # exp engines never drain at a boundary.
            pend = []
            pv_cur = None
            NP = NJT // 2
            stream = [(cc, h, p) for cc in range(NOC) for h in range(NH) for p in range(NP)]
            for idx, (cc, h, p) in enumerate(stream + [(None, None, q) for q in range(LA)]):
                tail = cc is None
                if not tail:
                    if p == 2 and deferred is not None:
                        deferred[0]()
                    if p == 6 and deferred is not None:
                        deferred[1]()
                        deferred = None
                    if p == 0:
                        if pv_cur is not None:
                            deferred2 = pv_cur
                        pv_cur = (pv_ps_pool.tile([D + 1, CW], f32, tag="pv", name="pv"), cc, h)
                    cslice = bass.ts(cc, CW)
                    # the pair's two K=64 matmuls are emitted back-to-back so
                    # they pack into disjoint PE row-groups (base_partition
                    # 0 / 64) and run concurrently in one PE pass; ONE
                    # pair-wide exp instruction then drains both halves.
                    pss = [sc_ps.tile([PDIM, CW], f32, tag="sc", name="sc") for _ in range(2)]
                    for s in range(2):
                        jt = 2 * p + s
                        nc.tensor.matmul(
                            out=pss[s],
                            lhsT=k_src(h, s)[s * D : (s + 1) * D, bass.ts(jt, PDIM)],
                            rhs=q_src(h, s)[s * D : (s + 1) * D, cslice],
                            start=True,
                            stop=True,
                        )
                    es = espool.tile([PDIM, 2 * CW], bf16, tag="es")
                    # CONCURRENT half-split: Scalar exps half s=0 while the
                    # Vector engine bit-exps half s=1 in parallel, so the
                    # score PSUM pair frees in one half-latency instead of a
                    # full pair-exp latency (shorter buffer round trip).
                    # Scalar also takes s=1 of two pairs per vchunk for load
                    # balance (18/14).
                    for s in range(2):
                        if s == 0 or p % 8 == 3:
                            # q pre-scaled by d^-0.5*log2(e) host-side
                            nc.scalar.activation(
                                out=es[:, bass.ts(s, CW)], in_=pss[s],
                                func=Exp, scale=0.6931471805599453,
                            )
                        else:
                            # 2^t via bf16 bit construction:
                            # int16(t*128+16256.5) == bf16 bits of 2^t
                            nc.vector.tensor_scalar(
                                es[:, bass.ts(s, CW)].bitcast(i16), pss[s],
                                128.0, 16256.5, MUL, ADD,
                            )
                    pend.append((pv_cur, p, es))
                while len(pend) > (0 if tail and p == LA - 1 else LA) or (tail and len(pend) > LA - 1 - p):
                    (pvt, pcc, ph), p0, es0 = pend.pop(0)
                    for s in range(2):
                        jt0 = 2 * p0 + s
                        nc.tensor.matmul(
                            out=pvt,
                            lhsT=v4[:, jt0, ph, :],
                            rhs=es0[:, bass.ts(s, CW)],
                            start=(jt0 == 0),
                            stop=(jt0 == NJT - 1),
                        )
                    if p0 == NP - 1:
                        deferred = make_chain(pcc, pvt)
            deferred[0]()
            deferred[1]()

            ps_stack2.close()

    nc.compile()
    return nc


def make_in_maps(x, gn_gamma, gn_beta, w_qkv, w_proj, b_proj, HW):
    """Per-core input dicts. Core c = (b = c//4, quarter s = c%4)."""
    import ml_dtypes

    bf16 = ml_dtypes.bfloat16
    OWN = HW // 4
    log2e = np.log2(np.e)
    x2 = np.ascontiguousarray(x.reshape(B, C, HW).astype(np.float32))
    w_qkv = np.asarray(w_qkv, dtype=np.float32)
    w_proj = np.asarray(w_proj, dtype=np.float32)
    indf = np.zeros((2, PDIM, G), dtype=np.float32)
    indb = np.zeros((2, G, PDIM), dtype=np.float32)
    gsz = C // G  # 32 channels per group
    for t in range(2):
        for p in range(PDIM):
            g = (t * PDIM + p) // gsz
            indf[t, p, g] = 1.0 / gsz
            indb[t, g, p] = 1.0
    wqT = np.ascontiguousarray(w_qkv[0:C, :].T * (D ** -0.5 * log2e)).astype(bf16)
    wkT = np.ascontiguousarray(w_qkv[C : 2 * C, :].T).astype(bf16)
    wvT = np.ascontiguousarray(w_qkv[2 * C : 3 * C, :].T).astype(bf16)
    wpT = np.ascontiguousarray(w_proj.T).astype(bf16)
    in_maps = []
    for c in range(NCORES):
        b, s = c // 4, c % 4
        in_maps.append(
            {
                "xb": x2[b],
                "x_own": np.ascontiguousarray(x2[b][:, s * OWN : (s + 1) * OWN]),
                "wqT": wqT,
                "wkT": wkT,
                "wvT": wvT,
                "wpT": wpT,
                "gamma": np.asarray(gn_gamma, dtype=np.float32),
                "beta": np.asarray(gn_beta, dtype=np.float32),
                "bproj": np.asarray(b_proj, dtype=np.float32),
                "indf": indf,
                "indb": indb,
            }
        )
    return in_maps


def assemble_output(results, HW, Himg, Wimg):
    OWN = HW // 4
    y = np.empty((B, C, HW), dtype=np.float32)
    for c in range(NCORES):
        b, s = c // 4, c % 4
        y[b][:, s * OWN : (s + 1) * OWN] = results[c]["y"]
    return y.reshape(B, C, Himg, Wimg)


_NC_CACHE = {}


def kernel(x, gn_gamma, gn_beta, w_qkv, w_proj, b_proj):
    from concourse.bass_utils import run_bass_kernel_spmd

    Himg, Wimg = x.shape[2], x.shape[3]
    HW = Himg * Wimg
    if HW not in _NC_CACHE:
        _NC_CACHE[HW] = build_nc(HW)
    nc = _NC_CACHE[HW]
    in_maps = make_in_maps(x, gn_gamma, gn_beta, w_qkv, w_proj, b_proj, HW)
    res = run_bass_kernel_spmd(nc, in_maps, list(range(NCORES)))
    return assemble_output(res.results, HW, Himg, Wimg)



# revision 11
# speedup vs baseline: 1.1356x; 1.1356x over previous
"""Trainium2 Bass kernel for an AttentionBlock (GroupNorm + single-layer MHA + proj residual).

Reference computation (per batch b):
    xn = GroupNorm(x[b])                        # 8 groups over C=256, HW spatial
    qkv = w_qkv @ xn                            # per-pixel 1x1 conv
    per head h (4 heads, d=64):
        scores = q_h^T k_h * d^-0.5             # [HW, HW]
        attn = softmax(scores, axis=keys)
        out_h = v_h @ attn^T                    # [d, HW]
    y = xn + w_proj @ concat(out_h) + b_proj

Sharding: 8 cores = (batch b in {0,1}) x (query quarter s in {0..3}).  Each
core runs GroupNorm, computes k/v for ALL spatial positions and q for its
own quarter, then runs all 4 heads' attention for its own 1024 query
columns.  The head sum of the projection is a local PSUM accumulation, so
there is NO collective at all: each core writes its own [C, 1024] slice of
the output, with the residual fused into the PSUM drain.

Key kernel-level layout choices (v2):
 - x columns are permuted host-side so each core's OWN quarter comes first;
   attention is permutation-invariant over keys, so k/v/score column order
   doesn't matter.  This kills the separate x_own load and lets the
   residual slice come straight out of the x/xn tiles.
 - scores are computed TRANSPOSED (keys j on partitions, queries i on the
   free axis); softmax denominator comes free as a 65th "ones" column of V.
 - softmax skips max-subtraction; scores live in the log2 domain (q
   pre-scaled by d^-0.5*log2 e host-side).
 - each score PAIR (2 key tiles x 512 queries) lands in ONE 2-bank PSUM
   tile [128,2,512]; ONE pair-wide exp instruction (Scalar native EXP or
   Vector int8 bit-trick) converts it to fp8e5 `es`.  e5m2's 4 steps/octave
   means the bit-trick value range is always a safe positive int8.
 - PV runs as a single fp8 DoubleRow matmul per pair (v4 fp8e4 stationary,
   es fp8e5 moving), halving PE time vs two bf16 matmuls and keeping the
   PE dense enough for the HAM clock gate to hold 2.4 GHz.
 - projection accumulates in a score-pool PSUM slot; residual fused in the
   drain.  Prologue: interleaved x-chunk DMAs (both halves round-robin) so
   GroupNorm stats finish right after the load; a couple of discarded f32
   matmuls on late x chunks pre-warm the PE clock.
"""

import numpy as np

C = 256
NH = 4
D = 64
G = 8
EPS = 1e-5
B = 2
NCORES = 8
PDIM = 128  # partitions
VP = 68     # v4 per-(jt,head) stride: 4*68=272 bytes, dual-fp8 ldweights needs %16==0

PREWARM = True
# per-vchunk exp engine pattern (16 pairs): S=scalar native exp, V=vector trick
EXP_PATTERN = "SVSVSVSSVSVSVSSV"


def build_nc(HW: int):
    import concourse.bass as bass
    import concourse.mybir as mybir
    import concourse.tile as tile
    from concourse import bacc

    f32 = mybir.dt.float32
    bf16 = mybir.dt.bfloat16
    fp8e4 = mybir.dt.float8e4
    fp8e5 = mybir.dt.float8e5
    i8 = mybir.dt.int8
    DR = mybir.MatmulPerfMode.DoubleRow
    CW = min(512, HW)          # i-chunk width (matmul moving-operand max)
    NIC = HW // CW             # number of column chunks of the full image
    OWN = HW // 4              # query columns owned per core
    NOC = OWN // CW            # own-column chunks
    NJT = HW // PDIM           # number of key tiles (128 keys each)
    NP = NJT // 2              # pairs of key tiles
    LA = 3                     # pv lookahead in pairs

    nc = bacc.Bacc(
        "TRN2", target_bir_lowering=False, debug=False, num_devices=NCORES
    )

    xb = nc.declare_dram_parameter("xb", [C, HW], f32, isOutput=False)
    wqT = nc.declare_dram_parameter("wqT", [C, C], bf16, isOutput=False)
    wkT = nc.declare_dram_parameter("wkT", [C, C], bf16, isOutput=False)
    wvT = nc.declare_dram_parameter("wvT", [C, C], bf16, isOutput=False)
    wpT = nc.declare_dram_parameter("wpT", [C, C], bf16, isOutput=False)
    gamma = nc.declare_dram_parameter("gamma", [C], f32, isOutput=False)
    beta = nc.declare_dram_parameter("beta", [C], f32, isOutput=False)
    bproj = nc.declare_dram_parameter("bproj", [C], f32, isOutput=False)
    indf = nc.declare_dram_parameter("indf", [2, PDIM, G], f32, isOutput=False)
    indb = nc.declare_dram_parameter("indb", [2, G, PDIM], f32, isOutput=False)
    y = nc.declare_dram_parameter("y", [C, OWN], f32, isOutput=True)

    Exp = mybir.ActivationFunctionType.Exp
    Sqrt = mybir.ActivationFunctionType.Sqrt
    Ident = mybir.ActivationFunctionType.Identity
    MUL = mybir.AluOpType.mult
    ADD = mybir.AluOpType.add

    BNW = min(512, HW)         # bn_stats max free dim
    NBN = HW // BNW
    LN2 = 0.6931471805599453

    with tile.TileContext(nc) as tc:
        with (
            tc.tile_pool(name="consts", bufs=1) as consts,
            tc.tile_pool(name="xpool", bufs=1) as xpool,
            tc.tile_pool(name="xnpool", bufs=1) as xnpool,
            tc.tile_pool(name="gn_sm", bufs=2) as gn_sm,
            tc.tile_pool(name="qkpool", bufs=1) as qkpool,
            tc.tile_pool(name="espool", bufs=6) as espool,
            tc.tile_pool(name="mlsm", bufs=3) as mlsm,
            tc.tile_pool(name="ypool", bufs=4) as ypool,
        ):
            # ---------------- x load (biggest transfer, gates GN) ----------------
            # Interleave the two channel-halves chunk-by-chunk across the three
            # DMA-capable queues so bn_stats for BOTH halves trail the load by
            # only one chunk.
            dma_engines = [nc.sync, nc.scalar, nc.gpsimd]
            x_sb = [
                xpool.tile([PDIM, HW], f32, tag=f"x{t}", name=f"x{t}") for t in range(2)
            ]
            di = 0
            for c in range(NIC):
                for t in range(2):
                    dma_engines[di % 3].dma_start(
                        out=x_sb[t][:, bass.ts(c, CW)],
                        in_=xb[bass.ts(t, PDIM), bass.ts(c, CW)],
                    )
                    di += 1

            # ---------------- constants / small loads ----------------
            eps_t = consts.tile([PDIM, 1], f32)
            nc.vector.memset(eps_t, EPS)
            nln2 = consts.tile([PDIM, 1], f32, tag="nln2")
            nc.vector.memset(nln2, -2.0 * 0.6931471805599453)

            indf_sb = []
            indb_sb = []
            gm_sb = []
            bt_sb = []
            bp_sb = []
            for t in range(2):
                it_ = consts.tile([PDIM, G], f32, tag=f"indf{t}")
                nc.sync.dma_start(out=it_, in_=indf[t])
                indf_sb.append(it_)
                ib_ = consts.tile([G, PDIM], f32, tag=f"indb{t}")
                nc.sync.dma_start(out=ib_, in_=indb[t])
                indb_sb.append(ib_)
                g_ = consts.tile([PDIM, 1], f32, tag=f"gm{t}")
                nc.sync.dma_start(out=g_, in_=gamma[bass.ts(t, PDIM)].rearrange("(p o) -> p o", o=1))
                gm_sb.append(g_)
                b_ = consts.tile([PDIM, 1], f32, tag=f"bt{t}")
                nc.sync.dma_start(out=b_, in_=beta[bass.ts(t, PDIM)].rearrange("(p o) -> p o", o=1))
                bt_sb.append(b_)
                bp_ = consts.tile([PDIM, 1], f32, tag=f"bp{t}")
                nc.sync.dma_start(out=bp_, in_=bproj[bass.ts(t, PDIM)].rearrange("(p o) -> p o", o=1))
                bp_sb.append(bp_)

            # weight tiles: [c-half t, 256 outputs] each; wp per head
            wq_sb, wk_sb, wv_sb = [], [], []
            for t in range(2):
                for (w_sb, src, tag) in (
                    (wq_sb, wqT, "wq"),
                    (wk_sb, wkT, "wk"),
                    (wv_sb, wvT, "wv"),
                ):
                    wt = consts.tile([PDIM, C], bf16, tag=f"{tag}{t}")
                    nc.sync.dma_start(out=wt, in_=src[bass.ts(t, PDIM), :])
                    w_sb.append(wt)
            wp_sb = []
            for h in range(NH):
                wt = consts.tile([D, C], bf16, tag=f"wp{h}", name=f"wp{h}")
                nc.sync.dma_start(out=wt, in_=wpT[h * D : (h + 1) * D, :])
                wp_sb.append(wt)

            from contextlib import ExitStack

            ps_stack = ExitStack()
            gn_ps = ps_stack.enter_context(tc.tile_pool(name="gn_ps", bufs=1, space="PSUM"))

            # ---------------- PE pre-warm (discarded f32 matmuls) ----------------
            # The PE HAM clock gate needs ~3.4us of sustained activity to release
            # 2.4 GHz.  Two slow f32 matmuls on late x chunks put the PE in the
            # busy state right before the GN/QKV/attention stream begins.
            if PREWARM:
                warm = gn_ps.tile([PDIM, 2, CW], f32, tag="warm")
                for w in range(2):
                    nc.tensor.matmul(
                        out=warm[:, w, :],
                        lhsT=x_sb[0][:, (NIC - 2 + w) * CW : (NIC - 2 + w) * CW + PDIM],
                        rhs=x_sb[1][:, bass.ts(NIC - 2 + w, CW)],
                        start=True,
                        stop=True,
                    )

            # ---------------- GroupNorm stats ----------------
            gst_full = gn_ps.tile([PDIM, 2], f32, tag="gnps")
            gst_ps = gst_full[0:G, :]
            for t in range(2):
                stats = gn_sm.tile([PDIM, NBN, 6], f32, tag="bnst")
                for s in range(NBN):
                    nc.vector.bn_stats(out=stats[:, s, :], in_=x_sb[t][:, bass.ts(s, BNW)])
                mv = gn_sm.tile([PDIM, 2], f32, tag="mv")
                nc.vector.bn_aggr(out=mv, in_=stats)
                st2 = gn_sm.tile([PDIM, 2], f32, tag="st2")
                nc.vector.tensor_copy(st2[:, 0:1], mv[:, 0:1])
                sq = gn_sm.tile([PDIM, 1], f32, tag="sq")
                nc.vector.tensor_mul(sq, mv[:, 0:1], mv[:, 0:1])
                nc.vector.tensor_add(st2[:, 1:2], mv[:, 1:2], sq)
                nc.tensor.matmul(
                    out=gst_ps, lhsT=indf_sb[t], rhs=st2, start=(t == 0), stop=(t == 1)
                )

            gst = gn_sm.tile([G, 2], f32, tag="gst_sb")
            nc.vector.tensor_copy(gst, gst_ps)
            mu2 = gn_sm.tile([G, 1], f32, tag="mu2")
            nc.vector.tensor_mul(mu2, gst[:, 0:1], gst[:, 0:1])
            var = gn_sm.tile([G, 1], f32, tag="var")
            nc.vector.tensor_sub(var, gst[:, 1:2], mu2)
            sd = gn_sm.tile([G, 1], f32, tag="sd")
            nc.scalar.activation(out=sd, in_=var, func=Sqrt, bias=eps_t[0:G, :], scale=1.0)
            rstd = gn_sm.tile([G, 1], f32, tag="rstd")
            nc.vector.reciprocal(out=rstd, in_=sd)
            gmr = gn_sm.tile([G, 2], f32, tag="gmr")
            nc.vector.tensor_copy(gmr[:, 0:1], gst[:, 0:1])
            nc.vector.tensor_copy(gmr[:, 1:2], rstd)

            # per-channel affine params + normalized x + residual slice
            xn_sb = []
            resid_sb = []
            for t in range(2):
                gb_ps = gn_ps.tile([PDIM, 2], f32, tag="gnps")
                nc.tensor.matmul(out=gb_ps, lhsT=indb_sb[t], rhs=gmr, start=True, stop=True)
                gb = gn_sm.tile([PDIM, 2], f32, tag="gb_sb")
                nc.vector.tensor_copy(gb, gb_ps)
                A_t = gn_sm.tile([PDIM, 1], f32, tag=f"A{t}")
                nc.vector.tensor_mul(A_t, gb[:, 1:2], gm_sb[t])
                tmp = gn_sm.tile([PDIM, 1], f32, tag="tmp")
                nc.vector.tensor_mul(tmp, gb[:, 0:1], A_t)
                B_t = gn_sm.tile([PDIM, 1], f32, tag=f"B{t}")
                nc.vector.tensor_sub(B_t, bt_sb[t], tmp)
                B2_t = gn_sm.tile([PDIM, 1], f32, tag=f"B2{t}")
                nc.vector.tensor_add(B2_t, B_t, bp_sb[t])

                xn_t = xnpool.tile([PDIM, HW], bf16, tag=f"xn{t}")
                # per-chunk ops split Scalar/DVE so the first qkv matmuls
                # start as soon as their xn slice exists
                for cc in range(NIC):
                    if cc % 2 == 0:
                        nc.scalar.activation(
                            out=xn_t[:, bass.ts(cc, CW)],
                            in_=x_sb[t][:, bass.ts(cc, CW)],
                            func=Ident, bias=B_t, scale=A_t,
                        )
                    else:
                        nc.vector.tensor_scalar(
                            xn_t[:, bass.ts(cc, CW)],
                            x_sb[t][:, bass.ts(cc, CW)],
                            A_t, B_t, MUL, ADD,
                        )
                xn_sb.append(xn_t)
                rs_t = xnpool.tile([PDIM, OWN], f32, tag=f"res{t}")
                nc.gpsimd.tensor_scalar(rs_t, x_sb[t][:, 0:OWN], A_t, B2_t, MUL, ADD)
                resid_sb.append(rs_t)

            ps_stack.close()  # release GN PSUM banks
            ps_stack = ExitStack()
            qk_ps = ps_stack.enter_context(tc.tile_pool(name="qk_ps", bufs=3, space="PSUM"))

            # ---------------- k, q, v production ----------------
            # k/q: [256 out-ch = 4 heads x 64, cols]; heads 0,1 in out-half 0.
            # Each PSUM tile holds TWO column chunks -> one big drain each.
            drain_engs = [nc.scalar, nc.vector]
            dei = 0

            def drain(dst, src):
                nonlocal dei
                eng = drain_engs[dei % 2]
                dei += 1
                if eng is nc.scalar:
                    eng.copy(dst, src)
                else:
                    eng.tensor_copy(dst, src)

            ku = [qkpool.tile([PDIM, HW], bf16, tag=f"ku{co}", name=f"ku{co}") for co in range(2)]
            for co in range(2):
                for c2 in range(NIC // 2):
                    ps = qk_ps.tile([PDIM, 2, CW], f32, tag="k2")
                    for s in range(2):
                        for t in range(2):
                            nc.tensor.matmul(
                                out=ps[:, s, :],
                                lhsT=wk_sb[t][:, bass.ts(co, PDIM)],
                                rhs=xn_sb[t][:, bass.ts(2 * c2 + s, CW)],
                                start=(t == 0),
                                stop=(t == 1),
                            )
                    drain(ku[co][:, bass.ts(c2, 2 * CW)], ps.rearrange("p s w -> p (s w)"))

            qu = [qkpool.tile([PDIM, OWN], bf16, tag=f"qu{co}", name=f"qu{co}") for co in range(2)]
            for co in range(2):
                ps = qk_ps.tile([PDIM, 2, CW], f32, tag="k2")
                for s in range(NOC):
                    for t in range(2):
                        nc.tensor.matmul(
                            out=ps[:, s, :],
                            lhsT=wq_sb[t][:, bass.ts(co, PDIM)],
                            rhs=xn_sb[t][:, bass.ts(s, CW)],
                            start=(t == 0),
                            stop=(t == 1),
                        )
                drain(qu[co], ps.rearrange("p s w -> p (s w)"))

            # v: per key-tile pair, all 4 heads at once -> strided into v4 slots.
            v4 = qkpool.tile([PDIM, NJT, NH, VP], fp8e4, tag="v4")
            nc.vector.memset(v4[:, :, :, D : D + 1], 1.0)

            for p in range(NP):
                ps = qk_ps.tile([PDIM, 2, C], f32, tag="v2", bufs=2)
                for s in range(2):
                    for t in range(2):
                        nc.tensor.matmul(
                            out=ps[:, s, :],
                            lhsT=xn_sb[t][:, bass.ts(2 * p + s, PDIM)],
                            rhs=wv_sb[t],
                            start=(t == 0),
                            stop=(t == 1),
                        )
                drain(
                    v4[:, 2 * p : 2 * p + 2, :, 0:D],
                    ps.rearrange("p s (h d) -> p s h d", h=NH),
                )

            # ---------------- swapped-halves companion tiles ----------------
            # Score pair packing needs each head's k/q in BOTH partition
            # halves.  ku/qu hold head 2cp in the lower half and head 2cp+1 in
            # the upper; ONE extra tile per pair with the halves swapped covers
            # the other slot of each head (SBUF-to-SBUF DMA, off the engines).
            # Chunked so early key tiles are available early.
            kx = [qkpool.tile([PDIM, HW], bf16, tag=f"kx{cp}", name=f"kx{cp}") for cp in range(2)]
            qx = [qkpool.tile([PDIM, OWN], bf16, tag=f"qx{cp}", name=f"qx{cp}") for cp in range(2)]
            for cp in range(2):
                for c in range(NIC):
                    eng = dma_engines[(cp * NIC + c) % 3]
                    eng.dma_start(out=kx[cp][0:D, bass.ts(c, CW)], in_=ku[cp][D : 2 * D, bass.ts(c, CW)])
                    eng.dma_start(out=kx[cp][D : 2 * D, bass.ts(c, CW)], in_=ku[cp][0:D, bass.ts(c, CW)])
                nc.scalar.dma_start(out=qx[cp][0:D, :], in_=qu[cp][D : 2 * D, :])
                nc.scalar.dma_start(out=qx[cp][D : 2 * D, :], in_=qu[cp][0:D, :])

            def k_src(h, s):
                # head h's k at partition half s
                return (ku if (h % 2) == s else kx)[h // 2]

            def q_src(h, s):
                return (qu if (h % 2) == s else qx)[h // 2]

            # ---------------- main attention loop ----------------
            ps_stack.close()  # release GN/QKV PSUM banks
            ps_stack2 = ExitStack()
            sc_ps = ps_stack2.enter_context(tc.tile_pool(name="sc_ps", bufs=3, space="PSUM"))
            pv_ps_pool = ps_stack2.enter_context(tc.tile_pool(name="pv_ps", bufs=2, space="PSUM"))

            # Per (i-chunk, head) "vchunk": 16 score-pair/exp/PV-DR steps,
            # pipelined LA pairs deep.  Each vchunk's normalization chain
            # (recip -> broadcast -> onorm) is DEFERRED into the next vchunk's
            # pair loop; the projection (4-head PSUM accumulation in a
            # score-pool slot + fused residual) emits once its chunk's 4
            # onorms exist.
            onorms_by_cc = [[] for _ in range(NOC)]

            def emit_proj(cc):
                cslice = bass.ts(cc, CW)
                for co in range(2):
                    pj = sc_ps.tile([PDIM, 2, CW], f32, tag="sc", name=f"pj{co}")
                    for h in range(NH):
                        nc.tensor.matmul(
                            out=pj[:, 0, :],
                            lhsT=wp_sb[h][:, bass.ts(co, PDIM)],
                            rhs=onorms_by_cc[cc][h],
                            start=(h == 0),
                            stop=(h == NH - 1),
                        )
                    yf = ypool.tile([PDIM, CW], f32, tag="yf", name="yf")
                    nc.vector.tensor_add(yf, pj[:, 0, :], resid_sb[co][:, cslice])
                    nc.sync.dma_start(out=y[bass.ts(co, PDIM), cslice], in_=yf)

            def make_chain(cc, pv):
                state = {}

                def stage1():
                    den = mlsm.tile([1, CW], f32, tag="den", name="den")
                    nc.vector.tensor_copy(den, pv[D : D + 1, :])
                    rden = mlsm.tile([1, CW], f32, tag="rden", name="rden")
                    nc.vector.reciprocal_approx_fast(out=rden, in_=den)
                    rdenb = mlsm.tile([1, CW], f32, tag="rdenb", name="rdenb")
                    nc.gpsimd.tensor_copy(rdenb, rden)
                    rdb = mlsm.tile([D, CW], f32, tag="rdb", name="rdb", bufs=2)
                    nc.gpsimd.partition_broadcast(rdb, rdenb[:, :])
                    state["rdb"] = rdb

                def stage2():
                    onorm = mlsm.tile([D, CW], bf16, tag="onorm", bufs=5, name="onorm")
                    nc.vector.tensor_mul(onorm, state["rdb"], pv[0:D, :])
                    onorms_by_cc[cc].append(onorm)
                    if len(onorms_by_cc[cc]) == NH:
                        emit_proj(cc)
                return stage1, stage2

            # One flat pair-stream over (chunk, head): the pend queue carries
            # ACROSS vchunk boundaries, so the final PV of one head interleaves
            # with the next head's first score/exp pairs and the exp engines
            # never drain at a boundary.  post_q holds the deferred norm-chain
            # stages, drained one per pair-step so they fill pipeline slack.
            pend = []
            post_q = []
            pv_cur = None
            stream = [(cc, h, p) for cc in range(NOC) for h in range(NH) for p in range(NP)]
            for idx, (cc, h, p) in enumerate(stream + [(None, None, q) for q in range(LA)]):
                tail = cc is None
                if not tail:
                    if p == 0:
                        pv_cur = (pv_ps_pool.tile([D + 1, CW], f32, tag="pv", name="pv"), cc, h)
                    cslice = bass.ts(cc, CW)
                    # the pair's two K=64 score matmuls are row-packed into
                    # disjoint PE row-groups (base_partition 0/64) and run
                    # concurrently in one PE pass, writing the two banks of
                    # ONE PSUM tile; a single pair-wide exp drains both.
                    sc = sc_ps.tile([PDIM, 2, CW], f32, tag="sc", name="sc")
                    for s in range(2):
                        jt = 2 * p + s
                        nc.tensor.matmul(
                            out=sc[:, s, :],
                            lhsT=k_src(h, s)[s * D : (s + 1) * D, bass.ts(jt, PDIM)],
                            rhs=q_src(h, s)[s * D : (s + 1) * D, cslice],
                            start=True,
                            stop=True,
                        )
                    es = espool.tile([PDIM, 2, CW], fp8e5, tag="es")
                    if EXP_PATTERN[p % len(EXP_PATTERN)] == "S":
                        # q pre-scaled by d^-0.5*log2(e) host-side: 2^t = exp(ln2*t)
                        nc.scalar.activation(out=es, in_=sc, func=Exp, scale=LN2)
                    else:
                        # 2^t as fp8e5 bits: int8(4t + 60.5); t in [-8.4, 8.4] always
                        # maps to [27, 94] -- never negative/NaN codes.  (The e4m3
                        # variant is UNSAFE: int8 in [-128,-1] hits fp8e4 NaN codes.)
                        nc.vector.tensor_scalar(es.bitcast(i8), sc, 4.0, 60.5, MUL, ADD)
                    pend.append((pv_cur, p, es))
                while len(pend) > (0 if tail and p == LA - 1 else LA) or (tail and len(pend) > LA - 1 - p):
                    (pvt, pcc, ph), p0, es0 = pend.pop(0)
                    nc.tensor.matmul(
                        out=pvt,
                        lhsT=v4[:, 2 * p0 : 2 * p0 + 2, ph, 0 : D + 1],
                        rhs=es0,
                        start=(p0 == 0),
                        stop=(p0 == NP - 1),
                        perf_mode=DR,
                    )
                    if p0 == NP - 1:
                        st1, st2 = make_chain(pcc, pvt)
                        post_q.append(st1)
                        post_q.append(st2)
                if post_q:
                    post_q.pop(0)()
            while post_q:
                post_q.pop(0)()

            ps_stack2.close()

    nc.compile()
    return nc


def make_in_maps(x, gn_gamma, gn_beta, w_qkv, w_proj, b_proj, HW):
    """Per-core input dicts. Core c = (b = c//4, quarter s = c%4).
    x columns are rotated so the core's own quarter comes first."""
    import ml_dtypes

    bf16 = ml_dtypes.bfloat16
    OWN = HW // 4
    log2e = np.log2(np.e)
    x2 = np.ascontiguousarray(x.reshape(B, C, HW).astype(np.float32))
    w_qkv = np.asarray(w_qkv, dtype=np.float32)
    w_proj = np.asarray(w_proj, dtype=np.float32)
    indf = np.zeros((2, PDIM, G), dtype=np.float32)
    indb = np.zeros((2, G, PDIM), dtype=np.float32)
    gsz = C // G  # 32 channels per group
    for t in range(2):
        for p in range(PDIM):
            g = (t * PDIM + p) // gsz
            indf[t, p, g] = 1.0 / gsz
            indb[t, g, p] = 1.0
    wqT = np.ascontiguousarray(w_qkv[0:C, :].T * (D ** -0.5 * log2e)).astype(bf16)
    wkT = np.ascontiguousarray(w_qkv[C : 2 * C, :].T).astype(bf16)
    wvT = np.ascontiguousarray(w_qkv[2 * C : 3 * C, :].T).astype(bf16)
    wpT = np.ascontiguousarray(w_proj.T).astype(bf16)
    in_maps = []
    for c in range(NCORES):
        b, s = c // 4, c % 4
        xrot = np.roll(x2[b], -s * OWN, axis=1)
        in_maps.append(
            {
                "xb": np.ascontiguousarray(xrot),
                "wqT": wqT,
                "wkT": wkT,
                "wvT": wvT,
                "wpT": wpT,
                "gamma": np.asarray(gn_gamma, dtype=np.float32),
                "beta": np.asarray(gn_beta, dtype=np.float32),
                "bproj": np.asarray(b_proj, dtype=np.float32),
                "indf": indf,
                "indb": indb,
            }
        )
    return in_maps


def assemble_output(results, HW, Himg, Wimg):
    OWN = HW // 4
    y = np.empty((B, C, HW), dtype=np.float32)
    for c in range(NCORES):
        b, s = c // 4, c % 4
        y[b][:, s * OWN : (s + 1) * OWN] = results[c]["y"]
    return y.reshape(B, C, Himg, Wimg)


_NC_CACHE = {}


def kernel(x, gn_gamma, gn_beta, w_qkv, w_proj, b_proj):
    from concourse.bass_utils import run_bass_kernel_spmd

    Himg, Wimg = x.shape[2], x.shape[3]
    HW = Himg * Wimg
    if HW not in _NC_CACHE:
        _NC_CACHE[HW] = build_nc(HW)
    nc = _NC_CACHE[HW]
    in_maps = make_in_maps(x, gn_gamma, gn_beta, w_qkv, w_proj, b_proj, HW)
    res = run_bass_kernel_spmd(nc, in_maps, list(range(NCORES)))
    return assemble_output(res.results, HW, Himg, Wimg)


# revision 12
# speedup vs baseline: 1.6074x; 1.4155x over previous
"""Trainium2 Bass kernel for an AttentionBlock (GroupNorm + single-layer MHA + proj residual).

Reference computation (per batch b):
    xn = GroupNorm(x[b])                        # 8 groups over C=256, HW spatial
    qkv = w_qkv @ xn                            # per-pixel 1x1 conv
    per head h (4 heads, d=64):
        scores = q_h^T k_h * d^-0.5             # [HW, HW]
        attn = softmax(scores, axis=keys)
        out_h = v_h @ attn^T                    # [d, HW]
    y = xn + w_proj @ concat(out_h) + b_proj

Sharding: 8 cores = (batch b in {0,1}) x (query quarter s in {0..3}).  Each
core runs GroupNorm, computes k/v for ALL spatial positions and q for its
own quarter, then runs all 4 heads' attention for its own 1024 query
columns.  The head sum of the projection is a local PSUM accumulation, so
there is NO collective at all: each core writes its own [C, 1024] slice of
the output, with the residual fused into the PSUM drain.

Key kernel-level layout choices (v2):
 - x columns are permuted host-side so each core's OWN quarter comes first;
   attention is permutation-invariant over keys, so k/v/score column order
   doesn't matter.  This kills the separate x_own load and lets the
   residual slice come straight out of the x/xn tiles.
 - scores are computed TRANSPOSED (keys j on partitions, queries i on the
   free axis); softmax denominator comes free as a 65th "ones" column of V.
 - softmax skips max-subtraction; scores live in the log2 domain (q
   pre-scaled by d^-0.5*log2 e host-side).
 - each score PAIR (2 key tiles x 512 queries) lands in ONE 2-bank PSUM
   tile [128,2,512]; ONE pair-wide exp instruction (Scalar native EXP or
   Vector int8 bit-trick) converts it to fp8e5 `es`.  e5m2's 4 steps/octave
   means the bit-trick value range is always a safe positive int8.
 - PV runs as a single fp8 DoubleRow matmul per pair (v4 fp8e4 stationary,
   es fp8e5 moving), halving PE time vs two bf16 matmuls and keeping the
   PE dense enough for the HAM clock gate to hold 2.4 GHz.
 - projection accumulates in a score-pool PSUM slot; residual fused in the
   drain.  Prologue: interleaved x-chunk DMAs (both halves round-robin) so
   GroupNorm stats finish right after the load; a couple of discarded f32
   matmuls on late x chunks pre-warm the PE clock.
"""

import numpy as np

C = 256
NH = 4
D = 64
G = 8
EPS = 1e-5
B = 2
NCORES = 8
PDIM = 128  # partitions
VP = 68     # v4 per-(jt,head) stride: 4*68=272 bytes, dual-fp8 ldweights needs %16==0

PREWARM = True
# per-vchunk exp engine pattern (16 pairs): S=scalar native exp, V=vector trick
EXP_PATTERN = "SVSVSVSSVSVSVSSV"


def build_nc(HW: int):
    import concourse.bass as bass
    import concourse.mybir as mybir
    import concourse.tile as tile
    from concourse import bacc

    f32 = mybir.dt.float32
    bf16 = mybir.dt.bfloat16
    fp8e4 = mybir.dt.float8e4
    fp8e5 = mybir.dt.float8e5
    i8 = mybir.dt.int8
    DR = mybir.MatmulPerfMode.DoubleRow
    CW = min(512, HW)          # i-chunk width (matmul moving-operand max)
    NIC = HW // CW             # number of column chunks of the full image
    OWN = HW // 4              # query columns owned per core
    NOC = OWN // CW            # own-column chunks
    NJT = HW // PDIM           # number of key tiles (128 keys each)
    NP = NJT // 2              # pairs of key tiles
    LA = 3                     # pv lookahead in pairs

    nc = bacc.Bacc(
        "TRN2", target_bir_lowering=False, debug=False, num_devices=NCORES
    )

    xb = nc.declare_dram_parameter("xb", [C, HW], f32, isOutput=False)
    wqT = nc.declare_dram_parameter("wqT", [C, C], bf16, isOutput=False)
    wkT = nc.declare_dram_parameter("wkT", [C, C], bf16, isOutput=False)
    wvT = nc.declare_dram_parameter("wvT", [C, C], bf16, isOutput=False)
    wpT = nc.declare_dram_parameter("wpT", [C, C], bf16, isOutput=False)
    gamma = nc.declare_dram_parameter("gamma", [C], f32, isOutput=False)
    beta = nc.declare_dram_parameter("beta", [C], f32, isOutput=False)
    bproj = nc.declare_dram_parameter("bproj", [C], f32, isOutput=False)
    indf = nc.declare_dram_parameter("indf", [2, PDIM, G], f32, isOutput=False)
    indb = nc.declare_dram_parameter("indb", [2, G, PDIM], f32, isOutput=False)
    y = nc.declare_dram_parameter("y", [C, OWN], f32, isOutput=True)

    Exp = mybir.ActivationFunctionType.Exp
    Sqrt = mybir.ActivationFunctionType.Sqrt
    Ident = mybir.ActivationFunctionType.Identity
    MUL = mybir.AluOpType.mult
    ADD = mybir.AluOpType.add

    BNW = min(512, HW)         # bn_stats max free dim
    NBN = HW // BNW
    LN2 = 0.6931471805599453

    with tile.TileContext(nc) as tc:
        with (
            tc.tile_pool(name="consts", bufs=1) as consts,
            tc.tile_pool(name="xpool", bufs=1) as xpool,
            tc.tile_pool(name="xnpool", bufs=1) as xnpool,
            tc.tile_pool(name="gn_sm", bufs=2) as gn_sm,
            tc.tile_pool(name="qkpool", bufs=1) as qkpool,
            tc.tile_pool(name="espool", bufs=6) as espool,
            tc.tile_pool(name="mlsm", bufs=3) as mlsm,
            tc.tile_pool(name="ypool", bufs=4) as ypool,
        ):
            # ---------------- x load (biggest transfer, gates GN) ----------------
            # Interleave the two channel-halves chunk-by-chunk across the three
            # DMA-capable queues so bn_stats for BOTH halves trail the load by
            # only one chunk.
            dma_engines = [nc.sync, nc.scalar, nc.gpsimd]
            x_sb = [
                xpool.tile([PDIM, HW], f32, tag=f"x{t}", name=f"x{t}") for t in range(2)
            ]
            di = 0
            for c in range(NIC):
                for t in range(2):
                    dma_engines[di % 3].dma_start(
                        out=x_sb[t][:, bass.ts(c, CW)],
                        in_=xb[bass.ts(t, PDIM), bass.ts(c, CW)],
                    )
                    di += 1

            # ---------------- constants / small loads ----------------
            eps_t = consts.tile([PDIM, 1], f32)
            nc.vector.memset(eps_t, EPS)
            nln2 = consts.tile([PDIM, 1], f32, tag="nln2")
            nc.vector.memset(nln2, -2.0 * 0.6931471805599453)

            indf_sb = []
            indb_sb = []
            gm_sb = []
            bt_sb = []
            bp_sb = []
            for t in range(2):
                it_ = consts.tile([PDIM, G], f32, tag=f"indf{t}")
                nc.sync.dma_start(out=it_, in_=indf[t])
                indf_sb.append(it_)
                ib_ = consts.tile([G, PDIM], f32, tag=f"indb{t}")
                nc.sync.dma_start(out=ib_, in_=indb[t])
                indb_sb.append(ib_)
                g_ = consts.tile([PDIM, 1], f32, tag=f"gm{t}")
                nc.sync.dma_start(out=g_, in_=gamma[bass.ts(t, PDIM)].rearrange("(p o) -> p o", o=1))
                gm_sb.append(g_)
                b_ = consts.tile([PDIM, 1], f32, tag=f"bt{t}")
                nc.sync.dma_start(out=b_, in_=beta[bass.ts(t, PDIM)].rearrange("(p o) -> p o", o=1))
                bt_sb.append(b_)
                bp_ = consts.tile([PDIM, 1], f32, tag=f"bp{t}")
                nc.sync.dma_start(out=bp_, in_=bproj[bass.ts(t, PDIM)].rearrange("(p o) -> p o", o=1))
                bp_sb.append(bp_)

            # weight tiles: [c-half t, 256 outputs] each; wp per head
            wq_sb, wk_sb, wv_sb = [], [], []
            for t in range(2):
                for (w_sb, src, tag) in (
                    (wq_sb, wqT, "wq"),
                    (wk_sb, wkT, "wk"),
                    (wv_sb, wvT, "wv"),
                ):
                    wt = consts.tile([PDIM, C], bf16, tag=f"{tag}{t}")
                    nc.sync.dma_start(out=wt, in_=src[bass.ts(t, PDIM), :])
                    w_sb.append(wt)
            wp_sb = []
            for h in range(NH):
                wt = consts.tile([D, C], bf16, tag=f"wp{h}", name=f"wp{h}")
                nc.sync.dma_start(out=wt, in_=wpT[h * D : (h + 1) * D, :])
                wp_sb.append(wt)

            from contextlib import ExitStack

            ps_stack = ExitStack()
            gn_ps = ps_stack.enter_context(tc.tile_pool(name="gn_ps", bufs=1, space="PSUM"))

            # ---------------- PE pre-warm (discarded f32 matmuls) ----------------
            # The PE HAM clock gate needs ~3.4us of sustained activity to release
            # 2.4 GHz.  Two slow f32 matmuls on late x chunks put the PE in the
            # busy state right before the GN/QKV/attention stream begins.
            if PREWARM:
                warm = gn_ps.tile([PDIM, 2, CW], f32, tag="warm")
                for w in range(2):
                    nc.tensor.matmul(
                        out=warm[:, w, :],
                        lhsT=x_sb[0][:, (NIC - 2 + w) * CW : (NIC - 2 + w) * CW + PDIM],
                        rhs=x_sb[1][:, bass.ts(NIC - 2 + w, CW)],
                        start=True,
                        stop=True,
                    )

            # ---------------- GroupNorm stats ----------------
            gst_full = gn_ps.tile([PDIM, 2], f32, tag="gnps")
            gst_ps = gst_full[0:G, :]
            for t in range(2):
                stats = gn_sm.tile([PDIM, NBN, 6], f32, tag="bnst")
                for s in range(NBN):
                    nc.vector.bn_stats(out=stats[:, s, :], in_=x_sb[t][:, bass.ts(s, BNW)])
                mv = gn_sm.tile([PDIM, 2], f32, tag="mv")
                nc.vector.bn_aggr(out=mv, in_=stats)
                st2 = gn_sm.tile([PDIM, 2], f32, tag="st2")
                nc.vector.tensor_copy(st2[:, 0:1], mv[:, 0:1])
                sq = gn_sm.tile([PDIM, 1], f32, tag="sq")
                nc.vector.tensor_mul(sq, mv[:, 0:1], mv[:, 0:1])
                nc.vector.tensor_add(st2[:, 1:2], mv[:, 1:2], sq)
                nc.tensor.matmul(
                    out=gst_ps, lhsT=indf_sb[t], rhs=st2, start=(t == 0), stop=(t == 1)
                )

            gst = gn_sm.tile([G, 2], f32, tag="gst_sb")
            nc.vector.tensor_copy(gst, gst_ps)
            mu2 = gn_sm.tile([G, 1], f32, tag="mu2")
            nc.vector.tensor_mul(mu2, gst[:, 0:1], gst[:, 0:1])
            var = gn_sm.tile([G, 1], f32, tag="var")
            nc.vector.tensor_sub(var, gst[:, 1:2], mu2)
            sd = gn_sm.tile([G, 1], f32, tag="sd")
            nc.scalar.activation(out=sd, in_=var, func=Sqrt, bias=eps_t[0:G, :], scale=1.0)
            rstd = gn_sm.tile([G, 1], f32, tag="rstd")
            nc.vector.reciprocal(out=rstd, in_=sd)
            gmr = gn_sm.tile([G, 2], f32, tag="gmr")
            nc.vector.tensor_copy(gmr[:, 0:1], gst[:, 0:1])
            nc.vector.tensor_copy(gmr[:, 1:2], rstd)

            # per-channel affine params + normalized x + residual slice
            xn_sb = []
            resid_sb = []
            for t in range(2):
                gb_ps = gn_ps.tile([PDIM, 2], f32, tag="gnps")
                nc.tensor.matmul(out=gb_ps, lhsT=indb_sb[t], rhs=gmr, start=True, stop=True)
                gb = gn_sm.tile([PDIM, 2], f32, tag="gb_sb")
                nc.vector.tensor_copy(gb, gb_ps)
                A_t = gn_sm.tile([PDIM, 1], f32, tag=f"A{t}")
                nc.vector.tensor_mul(A_t, gb[:, 1:2], gm_sb[t])
                tmp = gn_sm.tile([PDIM, 1], f32, tag="tmp")
                nc.vector.tensor_mul(tmp, gb[:, 0:1], A_t)
                B_t = gn_sm.tile([PDIM, 1], f32, tag=f"B{t}")
                nc.vector.tensor_sub(B_t, bt_sb[t], tmp)
                B2_t = gn_sm.tile([PDIM, 1], f32, tag=f"B2{t}")
                nc.vector.tensor_add(B2_t, B_t, bp_sb[t])

                xn_t = xnpool.tile([PDIM, HW], bf16, tag=f"xn{t}")
                # per-chunk ops split Scalar/DVE so the first qkv matmuls
                # start as soon as their xn slice exists
                for cc in range(NIC):
                    if cc % 2 == 0:
                        nc.scalar.activation(
                            out=xn_t[:, bass.ts(cc, CW)],
                            in_=x_sb[t][:, bass.ts(cc, CW)],
                            func=Ident, bias=B_t, scale=A_t,
                        )
                    else:
                        nc.vector.tensor_scalar(
                            xn_t[:, bass.ts(cc, CW)],
                            x_sb[t][:, bass.ts(cc, CW)],
                            A_t, B_t, MUL, ADD,
                        )
                xn_sb.append(xn_t)
                rs_t = xnpool.tile([PDIM, OWN], f32, tag=f"res{t}")
                nc.gpsimd.tensor_scalar(rs_t, x_sb[t][:, 0:OWN], A_t, B2_t, MUL, ADD)
                resid_sb.append(rs_t)

            ps_stack.close()  # release GN PSUM banks
            ps_stack = ExitStack()
            qk_ps = ps_stack.enter_context(tc.tile_pool(name="qk_ps", bufs=3, space="PSUM"))

            # pre-load the gpsimd partition_broadcast ucode lib while the PE/
            # engines are still in the prologue; the main loop's only gpsimd
            # compute is partition_broadcast, so the lib stays resident.
            warmbc = gn_sm.tile([D, G], f32, tag="warmbc")
            nc.gpsimd.partition_broadcast(warmbc, eps_t[0:1, :].broadcast_to([1, G]))

            # ---------------- k, q, v production ----------------
            # k/q: [256 out-ch = 4 heads x 64, cols]; heads 0,1 in out-half 0.
            # Each PSUM tile holds TWO column chunks -> one big drain each.
            drain_engs = [nc.scalar, nc.vector]
            dei = 0

            def drain(dst, src):
                nonlocal dei
                eng = drain_engs[dei % 2]
                dei += 1
                if eng is nc.scalar:
                    eng.copy(dst, src)
                else:
                    eng.tensor_copy(dst, src)

            ku = [qkpool.tile([PDIM, HW], bf16, tag=f"ku{co}", name=f"ku{co}") for co in range(2)]
            for co in range(2):
                for c2 in range(NIC // 2):
                    ps = qk_ps.tile([PDIM, 2, CW], f32, tag="k2")
                    for s in range(2):
                        for t in range(2):
                            nc.tensor.matmul(
                                out=ps[:, s, :],
                                lhsT=wk_sb[t][:, bass.ts(co, PDIM)],
                                rhs=xn_sb[t][:, bass.ts(2 * c2 + s, CW)],
                                start=(t == 0),
                                stop=(t == 1),
                            )
                    drain(ku[co][:, bass.ts(c2, 2 * CW)], ps.rearrange("p s w -> p (s w)"))

            qu = [qkpool.tile([PDIM, OWN], bf16, tag=f"qu{co}", name=f"qu{co}") for co in range(2)]
            for co in range(2):
                ps = qk_ps.tile([PDIM, 2, CW], f32, tag="k2")
                for s in range(NOC):
                    for t in range(2):
                        nc.tensor.matmul(
                            out=ps[:, s, :],
                            lhsT=wq_sb[t][:, bass.ts(co, PDIM)],
                            rhs=xn_sb[t][:, bass.ts(s, CW)],
                            start=(t == 0),
                            stop=(t == 1),
                        )
                drain(qu[co], ps.rearrange("p s w -> p (s w)"))

            # v: per key-tile pair, all 4 heads at once -> strided into v4 slots.
            v4 = qkpool.tile([PDIM, NJT, NH, VP], fp8e4, tag="v4")
            nc.vector.memset(v4[:, :, :, D : D + 1], 1.0)

            for p in range(NP):
                ps = qk_ps.tile([PDIM, 2, C], f32, tag="v2", bufs=2)
                for s in range(2):
                    for t in range(2):
                        nc.tensor.matmul(
                            out=ps[:, s, :],
                            lhsT=xn_sb[t][:, bass.ts(2 * p + s, PDIM)],
                            rhs=wv_sb[t],
                            start=(t == 0),
                            stop=(t == 1),
                        )
                drain(
                    v4[:, 2 * p : 2 * p + 2, :, 0:D],
                    ps.rearrange("p s (h d) -> p s h d", h=NH),
                )

            # ---------------- swapped-halves companion tiles ----------------
            # Score pair packing needs each head's k/q in BOTH partition
            # halves.  ku/qu hold head 2cp in the lower half and head 2cp+1 in
            # the upper; ONE extra tile per pair with the halves swapped covers
            # the other slot of each head (SBUF-to-SBUF DMA, off the engines).
            # Chunked so early key tiles are available early.
            kx = [qkpool.tile([PDIM, HW], bf16, tag=f"kx{cp}", name=f"kx{cp}") for cp in range(2)]
            qx = [qkpool.tile([PDIM, OWN], bf16, tag=f"qx{cp}", name=f"qx{cp}") for cp in range(2)]
            for cp in range(2):
                for c in range(NIC):
                    eng = dma_engines[(cp * NIC + c) % 3]
                    eng.dma_start(out=kx[cp][0:D, bass.ts(c, CW)], in_=ku[cp][D : 2 * D, bass.ts(c, CW)])
                    eng.dma_start(out=kx[cp][D : 2 * D, bass.ts(c, CW)], in_=ku[cp][0:D, bass.ts(c, CW)])
                nc.scalar.dma_start(out=qx[cp][0:D, :], in_=qu[cp][D : 2 * D, :])
                nc.scalar.dma_start(out=qx[cp][D : 2 * D, :], in_=qu[cp][0:D, :])

            def k_src(h, s):
                # head h's k at partition half s
                return (ku if (h % 2) == s else kx)[h // 2]

            def q_src(h, s):
                return (qu if (h % 2) == s else qx)[h // 2]

            # ---------------- main attention loop ----------------
            ps_stack.close()  # release GN/QKV PSUM banks
            ps_stack2 = ExitStack()
            sc_ps = ps_stack2.enter_context(tc.tile_pool(name="sc_ps", bufs=3, space="PSUM"))
            pv_ps_pool = ps_stack2.enter_context(tc.tile_pool(name="pv_ps", bufs=2, space="PSUM"))

            # Per (i-chunk, head) "vchunk": 16 score-pair/exp/PV-DR steps,
            # pipelined LA pairs deep.  Each vchunk's normalization chain
            # (recip -> broadcast -> onorm) is DEFERRED into the next vchunk's
            # pair loop; the projection (4-head PSUM accumulation in a
            # score-pool slot + fused residual) emits once its chunk's 4
            # onorms exist.
            onorms_by_cc = [[] for _ in range(NOC)]

            def emit_proj(cc):
                cslice = bass.ts(cc, CW)
                for co in range(2):
                    pj = sc_ps.tile([PDIM, 2, CW], f32, tag="sc", name=f"pj{co}")
                    for h in range(NH):
                        nc.tensor.matmul(
                            out=pj[:, 0, :],
                            lhsT=wp_sb[h][:, bass.ts(co, PDIM)],
                            rhs=onorms_by_cc[cc][h],
                            start=(h == 0),
                            stop=(h == NH - 1),
                        )
                    yf = ypool.tile([PDIM, CW], f32, tag="yf", name="yf")
                    nc.vector.tensor_add(yf, pj[:, 0, :], resid_sb[co][:, cslice])
                    nc.sync.dma_start(out=y[bass.ts(co, PDIM), cslice], in_=yf)

            def make_chain(cc, pv):
                state = {}

                def stage1():
                    den = mlsm.tile([1, CW], f32, tag="den", name="den")
                    nc.scalar.copy(den, pv[D : D + 1, :])
                    rden = mlsm.tile([1, CW], f32, tag="rden", name="rden")
                    nc.vector.reciprocal_approx_fast(out=rden, in_=den)
                    rdb = mlsm.tile([D, CW], f32, tag="rdb", name="rdb", bufs=2)
                    nc.gpsimd.partition_broadcast(rdb, rden[:, :])
                    state["rdb"] = rdb

                def stage2():
                    onorm = mlsm.tile([D, CW], bf16, tag="onorm", bufs=5, name="onorm")
                    nc.vector.tensor_mul(onorm, state["rdb"], pv[0:D, :])
                    onorms_by_cc[cc].append(onorm)
                    if len(onorms_by_cc[cc]) == NH:
                        emit_proj(cc)
                return stage1, stage2

            # One flat pair-stream over (chunk, head): the pend queue carries
            # ACROSS vchunk boundaries, so the final PV of one head interleaves
            # with the next head's first score/exp pairs and the exp engines
            # never drain at a boundary.  post_q holds the deferred norm-chain
            # stages, drained one per pair-step so they fill pipeline slack.
            pend = []
            post_q = []
            pv_cur = None
            stream = [(cc, h, p) for cc in range(NOC) for h in range(NH) for p in range(NP)]
            for idx, (cc, h, p) in enumerate(stream + [(None, None, q) for q in range(LA)]):
                tail = cc is None
                if not tail:
                    if p == 0:
                        pv_cur = (pv_ps_pool.tile([D + 1, CW], f32, tag="pv", name="pv"), cc, h)
                    cslice = bass.ts(cc, CW)
                    # the pair's two K=64 score matmuls are row-packed into
                    # disjoint PE row-groups (base_partition 0/64) and run
                    # concurrently in one PE pass, writing the two banks of
                    # ONE PSUM tile; a single pair-wide exp drains both.
                    sc = sc_ps.tile([PDIM, 2, CW], f32, tag="sc", name="sc")
                    for s in range(2):
                        jt = 2 * p + s
                        nc.tensor.matmul(
                            out=sc[:, s, :],
                            lhsT=k_src(h, s)[s * D : (s + 1) * D, bass.ts(jt, PDIM)],
                            rhs=q_src(h, s)[s * D : (s + 1) * D, cslice],
                            start=True,
                            stop=True,
                        )
                    es = espool.tile([PDIM, 2, CW], fp8e5, tag="es")
                    if EXP_PATTERN[p % len(EXP_PATTERN)] == "S":
                        # q pre-scaled by d^-0.5*log2(e) host-side: 2^t = exp(ln2*t)
                        nc.scalar.activation(out=es, in_=sc, func=Exp, scale=LN2)
                    else:
                        # 2^t as fp8e5 bits: int8(4t + 60.5); t in [-8.4, 8.4] always
                        # maps to [27, 94] -- never negative/NaN codes.  (The e4m3
                        # variant is UNSAFE: int8 in [-128,-1] hits fp8e4 NaN codes.)
                        nc.vector.tensor_scalar(es.bitcast(i8), sc, 4.0, 60.5, MUL, ADD)
                    pend.append((pv_cur, p, es))
                while len(pend) > (0 if tail and p == LA - 1 else LA) or (tail and len(pend) > LA - 1 - p):
                    (pvt, pcc, ph), p0, es0 = pend.pop(0)
                    nc.tensor.matmul(
                        out=pvt,
                        lhsT=v4[:, 2 * p0 : 2 * p0 + 2, ph, 0 : D + 1],
                        rhs=es0,
                        start=(p0 == 0),
                        stop=(p0 == NP - 1),
                        perf_mode=DR,
                    )
                    if p0 == NP - 1:
                        st1, st2 = make_chain(pcc, pvt)
                        post_q.append(st1)
                        post_q.append(st2)
                if post_q:
                    post_q.pop(0)()
            while post_q:
                post_q.pop(0)()

            ps_stack2.close()

    nc.compile()
    return nc


def make_in_maps(x, gn_gamma, gn_beta, w_qkv, w_proj, b_proj, HW):
    """Per-core input dicts. Core c = (b = c//4, quarter s = c%4).
    x columns are rotated so the core's own quarter comes first."""
    import ml_dtypes

    bf16 = ml_dtypes.bfloat16
    OWN = HW // 4
    log2e = np.log2(np.e)
    x2 = np.ascontiguousarray(x.reshape(B, C, HW).astype(np.float32))
    w_qkv = np.asarray(w_qkv, dtype=np.float32)
    w_proj = np.asarray(w_proj, dtype=np.float32)
    indf = np.zeros((2, PDIM, G), dtype=np.float32)
    indb = np.zeros((2, G, PDIM), dtype=np.float32)
    gsz = C // G  # 32 channels per group
    for t in range(2):
        for p in range(PDIM):
            g = (t * PDIM + p) // gsz
            indf[t, p, g] = 1.0 / gsz
            indb[t, g, p] = 1.0
    wqT = np.ascontiguousarray(w_qkv[0:C, :].T * (D ** -0.5 * log2e)).astype(bf16)
    wkT = np.ascontiguousarray(w_qkv[C : 2 * C, :].T).astype(bf16)
    wvT = np.ascontiguousarray(w_qkv[2 * C : 3 * C, :].T).astype(bf16)
    wpT = np.ascontiguousarray(w_proj.T).astype(bf16)
    in_maps = []
    for c in range(NCORES):
        b, s = c // 4, c % 4
        xrot = np.roll(x2[b], -s * OWN, axis=1)
        in_maps.append(
            {
                "xb": np.ascontiguousarray(xrot),
                "wqT": wqT,
                "wkT": wkT,
                "wvT": wvT,
                "wpT": wpT,
                "gamma": np.asarray(gn_gamma, dtype=np.float32),
                "beta": np.asarray(gn_beta, dtype=np.float32),
                "bproj": np.asarray(b_proj, dtype=np.float32),
                "indf": indf,
                "indb": indb,
            }
        )
    return in_maps


def assemble_output(results, HW, Himg, Wimg):
    OWN = HW // 4
    y = np.empty((B, C, HW), dtype=np.float32)
    for c in range(NCORES):
        b, s = c // 4, c % 4
        y[b][:, s * OWN : (s + 1) * OWN] = results[c]["y"]
    return y.reshape(B, C, Himg, Wimg)


_NC_CACHE = {}


def kernel(x, gn_gamma, gn_beta, w_qkv, w_proj, b_proj):
    from concourse.bass_utils import run_bass_kernel_spmd

    Himg, Wimg = x.shape[2], x.shape[3]
    HW = Himg * Wimg
    if HW not in _NC_CACHE:
        _NC_CACHE[HW] = build_nc(HW)
    nc = _NC_CACHE[HW]
    in_maps = make_in_maps(x, gn_gamma, gn_beta, w_qkv, w_proj, b_proj, HW)
    res = run_bass_kernel_spmd(nc, in_maps, list(range(NCORES)))
    return assemble_output(res.results, HW, Himg, Wimg)


# revision 13
# speedup vs baseline: 1.6194x; 1.0075x over previous
"""Trainium2 Bass kernel for an AttentionBlock (GroupNorm + single-layer MHA + proj residual).

Reference computation (per batch b):
    xn = GroupNorm(x[b])                        # 8 groups over C=256, HW spatial
    qkv = w_qkv @ xn                            # per-pixel 1x1 conv
    per head h (4 heads, d=64):
        scores = q_h^T k_h * d^-0.5             # [HW, HW]
        attn = softmax(scores, axis=keys)
        out_h = v_h @ attn^T                    # [d, HW]
    y = xn + w_proj @ concat(out_h) + b_proj

Sharding: 8 cores = (batch b in {0,1}) x (query quarter s in {0..3}).  Each
core runs GroupNorm, computes k/v for ALL spatial positions and q for its
own quarter, then runs all 4 heads' attention for its own 1024 query
columns.  The head sum of the projection is a local PSUM accumulation, so
there is NO collective at all: each core writes its own [C, 1024] slice of
the output, with the residual fused into the PSUM drain.

Key kernel-level layout choices (v2):
 - x columns are permuted host-side so each core's OWN quarter comes first;
   attention is permutation-invariant over keys, so k/v/score column order
   doesn't matter.  This kills the separate x_own load and lets the
   residual slice come straight out of the x/xn tiles.
 - scores are computed TRANSPOSED (keys j on partitions, queries i on the
   free axis); softmax denominator comes free as a 65th "ones" column of V.
 - softmax skips max-subtraction; scores live in the log2 domain (q
   pre-scaled by d^-0.5*log2 e host-side).
 - each score PAIR (2 key tiles x 512 queries) lands in ONE 2-bank PSUM
   tile [128,2,512]; ONE pair-wide exp instruction (Scalar native EXP or
   Vector int8 bit-trick) converts it to fp8e5 `es`.  e5m2's 4 steps/octave
   means the bit-trick value range is always a safe positive int8.
 - PV runs as a single fp8 DoubleRow matmul per pair (v4 fp8e4 stationary,
   es fp8e5 moving), halving PE time vs two bf16 matmuls and keeping the
   PE dense enough for the HAM clock gate to hold 2.4 GHz.
 - projection accumulates in a score-pool PSUM slot; residual fused in the
   drain.  Prologue: interleaved x-chunk DMAs (both halves round-robin) so
   GroupNorm stats finish right after the load; a couple of discarded f32
   matmuls on late x chunks pre-warm the PE clock.
"""

import numpy as np

C = 256
NH = 4
D = 64
G = 8
EPS = 1e-5
B = 2
NCORES = 8
PDIM = 128  # partitions
VP = 68     # v4 per-(jt,head) stride: 4*68=272 bytes, dual-fp8 ldweights needs %16==0

PREWARM = True
# per-vchunk exp engine pattern (16 pairs): S=scalar native exp, V=vector trick
EXP_PATTERN = "SVSVSVSSVSVSVSSV"


def build_nc(HW: int):
    import concourse.bass as bass
    import concourse.mybir as mybir
    import concourse.tile as tile
    from concourse import bacc

    f32 = mybir.dt.float32
    bf16 = mybir.dt.bfloat16
    fp8e4 = mybir.dt.float8e4
    fp8e5 = mybir.dt.float8e5
    i8 = mybir.dt.int8
    DR = mybir.MatmulPerfMode.DoubleRow
    CW = min(512, HW)          # i-chunk width (matmul moving-operand max)
    NIC = HW // CW             # number of column chunks of the full image
    OWN = HW // 4              # query columns owned per core
    NOC = OWN // CW            # own-column chunks
    NJT = HW // PDIM           # number of key tiles (128 keys each)
    NP = NJT // 2              # pairs of key tiles
    LA = 3                     # pv lookahead in pairs

    nc = bacc.Bacc(
        "TRN2", target_bir_lowering=False, debug=False, num_devices=NCORES
    )

    xb = nc.declare_dram_parameter("xb", [C, HW], f32, isOutput=False)
    wq4 = nc.declare_dram_parameter("wq4", [PDIM, 2, C], fp8e4, isOutput=False)
    wk4 = nc.declare_dram_parameter("wk4", [PDIM, 2, C], fp8e4, isOutput=False)
    wv4 = nc.declare_dram_parameter("wv4", [PDIM, 2, C], fp8e4, isOutput=False)
    wpT = nc.declare_dram_parameter("wpT", [C, C], bf16, isOutput=False)
    gamma = nc.declare_dram_parameter("gamma", [C], f32, isOutput=False)
    beta = nc.declare_dram_parameter("beta", [C], f32, isOutput=False)
    bproj = nc.declare_dram_parameter("bproj", [C], f32, isOutput=False)
    indf = nc.declare_dram_parameter("indf", [2, PDIM, G], f32, isOutput=False)
    indb = nc.declare_dram_parameter("indb", [2, G, PDIM], f32, isOutput=False)
    y = nc.declare_dram_parameter("y", [C, OWN], f32, isOutput=True)

    Exp = mybir.ActivationFunctionType.Exp
    Sqrt = mybir.ActivationFunctionType.Sqrt
    Ident = mybir.ActivationFunctionType.Identity
    MUL = mybir.AluOpType.mult
    ADD = mybir.AluOpType.add

    BNW = min(512, HW)         # bn_stats max free dim
    NBN = HW // BNW
    LN2 = 0.6931471805599453

    with tile.TileContext(nc) as tc:
        with (
            tc.tile_pool(name="consts", bufs=1) as consts,
            tc.tile_pool(name="xpool", bufs=1) as xpool,
            tc.tile_pool(name="xnpool", bufs=1) as xnpool,
            tc.tile_pool(name="gn_sm", bufs=2) as gn_sm,
            tc.tile_pool(name="qkpool", bufs=1) as qkpool,
            tc.tile_pool(name="espool", bufs=6) as espool,
            tc.tile_pool(name="mlsm", bufs=3) as mlsm,
            tc.tile_pool(name="ypool", bufs=4) as ypool,
        ):
            # ---------------- x load (biggest transfer, gates GN) ----------------
            # Interleave the two channel-halves chunk-by-chunk across the three
            # DMA-capable queues so bn_stats for BOTH halves trail the load by
            # only one chunk.
            dma_engines = [nc.sync, nc.scalar, nc.gpsimd]
            x_sb = [
                xpool.tile([PDIM, HW], f32, tag=f"x{t}", name=f"x{t}") for t in range(2)
            ]
            di = 0
            for c in range(NIC):
                for t in range(2):
                    dma_engines[di % 3].dma_start(
                        out=x_sb[t][:, bass.ts(c, CW)],
                        in_=xb[bass.ts(t, PDIM), bass.ts(c, CW)],
                    )
                    di += 1

            # ---------------- constants / small loads ----------------
            eps_t = consts.tile([PDIM, 1], f32)
            nc.vector.memset(eps_t, EPS)
            nln2 = consts.tile([PDIM, 1], f32, tag="nln2")
            nc.vector.memset(nln2, -2.0 * 0.6931471805599453)

            indf_sb = []
            indb_sb = []
            gm_sb = []
            bt_sb = []
            bp_sb = []
            for t in range(2):
                it_ = consts.tile([PDIM, G], f32, tag=f"indf{t}")
                nc.sync.dma_start(out=it_, in_=indf[t])
                indf_sb.append(it_)
                ib_ = consts.tile([G, PDIM], f32, tag=f"indb{t}")
                nc.sync.dma_start(out=ib_, in_=indb[t])
                indb_sb.append(ib_)
                g_ = consts.tile([PDIM, 1], f32, tag=f"gm{t}")
                nc.sync.dma_start(out=g_, in_=gamma[bass.ts(t, PDIM)].rearrange("(p o) -> p o", o=1))
                gm_sb.append(g_)
                b_ = consts.tile([PDIM, 1], f32, tag=f"bt{t}")
                nc.sync.dma_start(out=b_, in_=beta[bass.ts(t, PDIM)].rearrange("(p o) -> p o", o=1))
                bt_sb.append(b_)
                bp_ = consts.tile([PDIM, 1], f32, tag=f"bp{t}")
                nc.sync.dma_start(out=bp_, in_=bproj[bass.ts(t, PDIM)].rearrange("(p o) -> p o", o=1))
                bp_sb.append(bp_)

            # weight tiles: fp8, [p, c-half, 256 outputs] (DR k-subtile layout)
            wq_t = consts.tile([PDIM, 2, C], fp8e4, tag="wq")
            nc.sync.dma_start(out=wq_t, in_=wq4[:, :, :])
            wk_t = consts.tile([PDIM, 2, C], fp8e4, tag="wk")
            nc.sync.dma_start(out=wk_t, in_=wk4[:, :, :])
            wv_t = consts.tile([PDIM, 2, C], fp8e4, tag="wv")
            nc.sync.dma_start(out=wv_t, in_=wv4[:, :, :])
            wp_sb = []
            for h in range(NH):
                wt = consts.tile([D, C], bf16, tag=f"wp{h}", name=f"wp{h}")
                nc.sync.dma_start(out=wt, in_=wpT[h * D : (h + 1) * D, :])
                wp_sb.append(wt)

            from contextlib import ExitStack

            ps_stack = ExitStack()
            gn_ps = ps_stack.enter_context(tc.tile_pool(name="gn_ps", bufs=1, space="PSUM"))

            # ---------------- PE pre-warm (discarded f32 matmuls) ----------------
            # The PE HAM clock gate needs ~3.4us of sustained activity to release
            # 2.4 GHz.  Two slow f32 matmuls on late x chunks put the PE in the
            # busy state right before the GN/QKV/attention stream begins.
            if PREWARM:
                warm = gn_ps.tile([PDIM, 2, CW], f32, tag="warm")
                for w in range(4):
                    nc.tensor.matmul(
                        out=warm[:, w % 2, :],
                        lhsT=x_sb[0][:, (NIC - 4 + w) * CW : (NIC - 4 + w) * CW + PDIM],
                        rhs=x_sb[1][:, bass.ts(NIC - 4 + w, CW)],
                        start=True,
                        stop=True,
                    )

            # ---------------- GroupNorm stats ----------------
            gst_full = gn_ps.tile([PDIM, 2], f32, tag="gnps")
            gst_ps = gst_full[0:G, :]
            for t in range(2):
                stats = gn_sm.tile([PDIM, NBN, 6], f32, tag="bnst")
                for s in range(NBN):
                    nc.vector.bn_stats(out=stats[:, s, :], in_=x_sb[t][:, bass.ts(s, BNW)])
                mv = gn_sm.tile([PDIM, 2], f32, tag="mv")
                nc.vector.bn_aggr(out=mv, in_=stats)
                st2 = gn_sm.tile([PDIM, 2], f32, tag="st2")
                nc.vector.tensor_copy(st2[:, 0:1], mv[:, 0:1])
                sq = gn_sm.tile([PDIM, 1], f32, tag="sq")
                nc.vector.tensor_mul(sq, mv[:, 0:1], mv[:, 0:1])
                nc.vector.tensor_add(st2[:, 1:2], mv[:, 1:2], sq)
                nc.tensor.matmul(
                    out=gst_ps, lhsT=indf_sb[t], rhs=st2, start=(t == 0), stop=(t == 1)
                )

            gst = gn_sm.tile([G, 2], f32, tag="gst_sb")
            nc.vector.tensor_copy(gst, gst_ps)
            mu2 = gn_sm.tile([G, 1], f32, tag="mu2")
            nc.vector.tensor_mul(mu2, gst[:, 0:1], gst[:, 0:1])
            var = gn_sm.tile([G, 1], f32, tag="var")
            nc.vector.tensor_sub(var, gst[:, 1:2], mu2)
            sd = gn_sm.tile([G, 1], f32, tag="sd")
            nc.scalar.activation(out=sd, in_=var, func=Sqrt, bias=eps_t[0:G, :], scale=1.0)
            rstd = gn_sm.tile([G, 1], f32, tag="rstd")
            nc.vector.reciprocal(out=rstd, in_=sd)
            gmr = gn_sm.tile([G, 2], f32, tag="gmr")
            nc.vector.tensor_copy(gmr[:, 0:1], gst[:, 0:1])
            nc.vector.tensor_copy(gmr[:, 1:2], rstd)

            # per-channel affine params + normalized x + residual slice
            xn4 = xnpool.tile([PDIM, 2, HW], fp8e4, tag="xn4")
            resid_sb = []
            for t in range(2):
                gb_ps = gn_ps.tile([PDIM, 2], f32, tag="gnps")
                nc.tensor.matmul(out=gb_ps, lhsT=indb_sb[t], rhs=gmr, start=True, stop=True)
                gb = gn_sm.tile([PDIM, 2], f32, tag="gb_sb")
                nc.vector.tensor_copy(gb, gb_ps)
                A_t = gn_sm.tile([PDIM, 1], f32, tag=f"A{t}")
                nc.vector.tensor_mul(A_t, gb[:, 1:2], gm_sb[t])
                tmp = gn_sm.tile([PDIM, 1], f32, tag="tmp")
                nc.vector.tensor_mul(tmp, gb[:, 0:1], A_t)
                B_t = gn_sm.tile([PDIM, 1], f32, tag=f"B{t}")
                nc.vector.tensor_sub(B_t, bt_sb[t], tmp)
                B2_t = gn_sm.tile([PDIM, 1], f32, tag=f"B2{t}")
                nc.vector.tensor_add(B2_t, B_t, bp_sb[t])

                A16 = gn_sm.tile([PDIM, 1], f32, tag=f"A16{t}")
                nc.vector.tensor_scalar(A16, A_t, 16.0, 0.0, MUL, ADD)
                B16 = gn_sm.tile([PDIM, 1], f32, tag=f"B16{t}")
                nc.vector.tensor_scalar(B16, B_t, 16.0, 0.0, MUL, ADD)
                # 16*xn in fp8e4 (|16 xn| <~ 100 << 240); per-chunk ops split
                # Scalar/DVE so the first qkv matmuls start early
                for cc in range(NIC):
                    if cc % 2 == 0:
                        nc.scalar.activation(
                            out=xn4[:, t, bass.ts(cc, CW)],
                            in_=x_sb[t][:, bass.ts(cc, CW)],
                            func=Ident, bias=B16, scale=A16,
                        )
                    else:
                        nc.vector.tensor_scalar(
                            xn4[:, t, bass.ts(cc, CW)],
                            x_sb[t][:, bass.ts(cc, CW)],
                            A16, B16, MUL, ADD,
                        )
                rs_t = xnpool.tile([PDIM, OWN], f32, tag=f"res{t}")
                nc.gpsimd.tensor_scalar(rs_t, x_sb[t][:, 0:OWN], A_t, B2_t, MUL, ADD)
                resid_sb.append(rs_t)

            ps_stack.close()  # release GN PSUM banks
            ps_stack = ExitStack()
            qk_ps = ps_stack.enter_context(tc.tile_pool(name="qk_ps", bufs=3, space="PSUM"))

            # pre-load the gpsimd partition_broadcast ucode lib while the PE/
            # engines are still in the prologue; the main loop's only gpsimd
            # compute is partition_broadcast, so the lib stays resident.
            warmbc = gn_sm.tile([D, G], f32, tag="warmbc")
            nc.gpsimd.partition_broadcast(warmbc, eps_t[0:1, :].broadcast_to([1, G]))

            # ---------------- k, q, v production ----------------
            # k/q: [256 out-ch = 4 heads x 64, cols]; heads 0,1 in out-half 0.
            # Each PSUM tile holds TWO column chunks -> one big drain each.
            drain_engs = [nc.scalar, nc.vector]
            dei = 0

            def drain(dst, src):
                # qkv ran on 16x-scaled fp8 operands: descale by 1/256
                nonlocal dei
                eng = drain_engs[dei % 2]
                dei += 1
                if eng is nc.scalar:
                    eng.activation(out=dst, in_=src, func=Ident, bias=0.0, scale=1.0 / 256.0)
                else:
                    eng.tensor_scalar(dst, src, 1.0 / 256.0, 0.0, MUL, ADD)

            ku = [qkpool.tile([PDIM, HW], bf16, tag=f"ku{co}", name=f"ku{co}") for co in range(2)]
            for co in range(2):
                for c2 in range(NIC // 2):
                    ps = qk_ps.tile([PDIM, 2, CW], f32, tag="k2")
                    for s in range(2):
                        nc.tensor.matmul(
                            out=ps[:, s, :],
                            lhsT=wk_t[:, :, bass.ts(co, PDIM)],
                            rhs=xn4[:, :, bass.ts(2 * c2 + s, CW)],
                            start=True,
                            stop=True,
                            perf_mode=DR,
                        )
                    drain(ku[co][:, bass.ts(c2, 2 * CW)], ps.rearrange("p s w -> p (s w)"))

            qu = [qkpool.tile([PDIM, OWN], bf16, tag=f"qu{co}", name=f"qu{co}") for co in range(2)]
            for co in range(2):
                ps = qk_ps.tile([PDIM, 2, CW], f32, tag="k2")
                for s in range(NOC):
                    nc.tensor.matmul(
                        out=ps[:, s, :],
                        lhsT=wq_t[:, :, bass.ts(co, PDIM)],
                        rhs=xn4[:, :, bass.ts(s, CW)],
                        start=True,
                        stop=True,
                        perf_mode=DR,
                    )
                drain(qu[co], ps.rearrange("p s w -> p (s w)"))

            # v: per key-tile pair, all 4 heads at once -> strided into v4 slots.
            v4 = qkpool.tile([PDIM, NJT, NH, VP], fp8e4, tag="v4")
            nc.vector.memset(v4[:, :, :, D : D + 1], 1.0)

            for p in range(NP):
                ps = qk_ps.tile([PDIM, 2, C], f32, tag="v2", bufs=2)
                for s in range(2):
                    for t in range(2):
                        nc.tensor.matmul(
                            out=ps[:, s, :],
                            lhsT=xn4[:, t, bass.ts(2 * p + s, PDIM)],
                            rhs=wv_t[:, t, :],
                            start=(t == 0),
                            stop=(t == 1),
                        )
                drain(
                    v4[:, 2 * p : 2 * p + 2, :, 0:D],
                    ps.rearrange("p s (h d) -> p s h d", h=NH),
                )

            # ---------------- swapped-halves companion tiles ----------------
            # Score pair packing needs each head's k/q in BOTH partition
            # halves.  ku/qu hold head 2cp in the lower half and head 2cp+1 in
            # the upper; ONE extra tile per pair with the halves swapped covers
            # the other slot of each head (SBUF-to-SBUF DMA, off the engines).
            # Chunked so early key tiles are available early.
            kx = [qkpool.tile([PDIM, HW], bf16, tag=f"kx{cp}", name=f"kx{cp}") for cp in range(2)]
            qx = [qkpool.tile([PDIM, OWN], bf16, tag=f"qx{cp}", name=f"qx{cp}") for cp in range(2)]
            for cp in range(2):
                for c in range(NIC):
                    eng = dma_engines[(cp * NIC + c) % 3]
                    eng.dma_start(out=kx[cp][0:D, bass.ts(c, CW)], in_=ku[cp][D : 2 * D, bass.ts(c, CW)])
                    eng.dma_start(out=kx[cp][D : 2 * D, bass.ts(c, CW)], in_=ku[cp][0:D, bass.ts(c, CW)])
                nc.scalar.dma_start(out=qx[cp][0:D, :], in_=qu[cp][D : 2 * D, :])
                nc.scalar.dma_start(out=qx[cp][D : 2 * D, :], in_=qu[cp][0:D, :])

            def k_src(h, s):
                # head h's k at partition half s
                return (ku if (h % 2) == s else kx)[h // 2]

            def q_src(h, s):
                return (qu if (h % 2) == s else qx)[h // 2]

            # ---------------- main attention loop ----------------
            ps_stack.close()  # release GN/QKV PSUM banks
            ps_stack2 = ExitStack()
            sc_ps = ps_stack2.enter_context(tc.tile_pool(name="sc_ps", bufs=3, space="PSUM"))
            pv_ps_pool = ps_stack2.enter_context(tc.tile_pool(name="pv_ps", bufs=2, space="PSUM"))

            # Per (i-chunk, head) "vchunk": 16 score-pair/exp/PV-DR steps,
            # pipelined LA pairs deep.  Each vchunk's normalization chain
            # (recip -> broadcast -> onorm) is DEFERRED into the next vchunk's
            # pair loop; the projection (4-head PSUM accumulation in a
            # score-pool slot + fused residual) emits once its chunk's 4
            # onorms exist.
            onorms_by_cc = [[] for _ in range(NOC)]

            def emit_proj(cc):
                cslice = bass.ts(cc, CW)
                for co in range(2):
                    pj = sc_ps.tile([PDIM, 2, CW], f32, tag="sc", name=f"pj{co}")
                    for h in range(NH):
                        nc.tensor.matmul(
                            out=pj[:, 0, :],
                            lhsT=wp_sb[h][:, bass.ts(co, PDIM)],
                            rhs=onorms_by_cc[cc][h],
                            start=(h == 0),
                            stop=(h == NH - 1),
                        )
                    yf = ypool.tile([PDIM, CW], f32, tag="yf", name="yf")
                    nc.vector.tensor_add(yf, pj[:, 0, :], resid_sb[co][:, cslice])
                    nc.sync.dma_start(out=y[bass.ts(co, PDIM), cslice], in_=yf)

            def make_chain(cc, pv):
                state = {}

                def stage1():
                    den = mlsm.tile([1, CW], f32, tag="den", name="den")
                    nc.scalar.copy(den, pv[D : D + 1, :])
                    rden = mlsm.tile([1, CW], f32, tag="rden", name="rden")
                    nc.vector.reciprocal_approx_fast(out=rden, in_=den)
                    rdb = mlsm.tile([D, CW], f32, tag="rdb", name="rdb", bufs=2)
                    nc.gpsimd.partition_broadcast(rdb, rden[:, :])
                    state["rdb"] = rdb

                def stage2():
                    onorm = mlsm.tile([D, CW], bf16, tag="onorm", bufs=5, name="onorm")
                    nc.vector.tensor_mul(onorm, state["rdb"], pv[0:D, :])
                    onorms_by_cc[cc].append(onorm)
                    if len(onorms_by_cc[cc]) == NH:
                        emit_proj(cc)
                return stage1, stage2

            # One flat pair-stream over (chunk, head): the pend queue carries
            # ACROSS vchunk boundaries, so the final PV of one head interleaves
            # with the next head's first score/exp pairs and the exp engines
            # never drain at a boundary.  post_q holds the deferred norm-chain
            # stages, drained one per pair-step so they fill pipeline slack.
            pend = []
            post_q = []
            pv_cur = None
            stream = [(cc, h, p) for cc in range(NOC) for h in range(NH) for p in range(NP)]
            for idx, (cc, h, p) in enumerate(stream + [(None, None, q) for q in range(LA)]):
                tail = cc is None
                if not tail:
                    if p == 0:
                        pv_cur = (pv_ps_pool.tile([D + 1, CW], f32, tag="pv", name="pv"), cc, h)
                    cslice = bass.ts(cc, CW)
                    # the pair's two K=64 score matmuls are row-packed into
                    # disjoint PE row-groups (base_partition 0/64) and run
                    # concurrently in one PE pass, writing the two banks of
                    # ONE PSUM tile; a single pair-wide exp drains both.
                    sc = sc_ps.tile([PDIM, 2, CW], f32, tag="sc", name="sc")
                    for s in range(2):
                        jt = 2 * p + s
                        nc.tensor.matmul(
                            out=sc[:, s, :],
                            lhsT=k_src(h, s)[s * D : (s + 1) * D, bass.ts(jt, PDIM)],
                            rhs=q_src(h, s)[s * D : (s + 1) * D, cslice],
                            start=True,
                            stop=True,
                        )
                    es = espool.tile([PDIM, 2, CW], fp8e5, tag="es")
                    if EXP_PATTERN[p % len(EXP_PATTERN)] == "S":
                        # q pre-scaled by d^-0.5*log2(e) host-side: 2^t = exp(ln2*t)
                        nc.scalar.activation(out=es, in_=sc, func=Exp, scale=LN2)
                    else:
                        # 2^t as fp8e5 bits: int8(4t + 60.5); t in [-8.4, 8.4] always
                        # maps to [27, 94] -- never negative/NaN codes.  (The e4m3
                        # variant is UNSAFE: int8 in [-128,-1] hits fp8e4 NaN codes.)
                        nc.vector.tensor_scalar(es.bitcast(i8), sc, 4.0, 60.5, MUL, ADD)
                    pend.append((pv_cur, p, es))
                while len(pend) > (0 if tail and p == LA - 1 else LA) or (tail and len(pend) > LA - 1 - p):
                    (pvt, pcc, ph), p0, es0 = pend.pop(0)
                    nc.tensor.matmul(
                        out=pvt,
                        lhsT=v4[:, 2 * p0 : 2 * p0 + 2, ph, 0 : D + 1],
                        rhs=es0,
                        start=(p0 == 0),
                        stop=(p0 == NP - 1),
                        perf_mode=DR,
                    )
                    if p0 == NP - 1:
                        st1, st2 = make_chain(pcc, pvt)
                        post_q.append(st1)
                        post_q.append(st2)
                if post_q:
                    post_q.pop(0)()
            while post_q:
                post_q.pop(0)()

            ps_stack2.close()

    nc.compile()
    return nc


def make_in_maps(x, gn_gamma, gn_beta, w_qkv, w_proj, b_proj, HW):
    """Per-core input dicts. Core c = (b = c//4, quarter s = c%4).
    x columns are rotated so the core's own quarter comes first."""
    import ml_dtypes

    bf16 = ml_dtypes.bfloat16
    OWN = HW // 4
    log2e = np.log2(np.e)
    x2 = np.ascontiguousarray(x.reshape(B, C, HW).astype(np.float32))
    w_qkv = np.asarray(w_qkv, dtype=np.float32)
    w_proj = np.asarray(w_proj, dtype=np.float32)
    indf = np.zeros((2, PDIM, G), dtype=np.float32)
    indb = np.zeros((2, G, PDIM), dtype=np.float32)
    gsz = C // G  # 32 channels per group
    for t in range(2):
        for p in range(PDIM):
            g = (t * PDIM + p) // gsz
            indf[t, p, g] = 1.0 / gsz
            indb[t, g, p] = 1.0
    fp8 = ml_dtypes.float8_e4m3

    def w4(wslice, scale):
        # [C_in, C_out] -> [128, 2, C_out] fp8, x16 (qkv runs on 16x operands)
        wT = wslice.T * scale
        return np.ascontiguousarray(wT.reshape(2, PDIM, C).transpose(1, 0, 2)).astype(fp8)

    wq4 = w4(w_qkv[0:C, :], 16.0 * (D ** -0.5 * log2e))
    wk4 = w4(w_qkv[C : 2 * C, :], 16.0)
    wv4 = w4(w_qkv[2 * C : 3 * C, :], 16.0)
    wpT = np.ascontiguousarray(w_proj.T).astype(bf16)
    in_maps = []
    for c in range(NCORES):
        b, s = c // 4, c % 4
        xrot = np.roll(x2[b], -s * OWN, axis=1)
        in_maps.append(
            {
                "xb": np.ascontiguousarray(xrot),
                "wq4": wq4,
                "wk4": wk4,
                "wv4": wv4,
                "wpT": wpT,
                "gamma": np.asarray(gn_gamma, dtype=np.float32),
                "beta": np.asarray(gn_beta, dtype=np.float32),
                "bproj": np.asarray(b_proj, dtype=np.float32),
                "indf": indf,
                "indb": indb,
            }
        )
    return in_maps


def assemble_output(results, HW, Himg, Wimg):
    OWN = HW // 4
    y = np.empty((B, C, HW), dtype=np.float32)
    for c in range(NCORES):
        b, s = c // 4, c % 4
        y[b][:, s * OWN : (s + 1) * OWN] = results[c]["y"]
    return y.reshape(B, C, Himg, Wimg)


_NC_CACHE = {}


def kernel(x, gn_gamma, gn_beta, w_qkv, w_proj, b_proj):
    from concourse.bass_utils import run_bass_kernel_spmd

    Himg, Wimg = x.shape[2], x.shape[3]
    HW = Himg * Wimg
    if HW not in _NC_CACHE:
        _NC_CACHE[HW] = build_nc(HW)
    nc = _NC_CACHE[HW]
    in_maps = make_in_maps(x, gn_gamma, gn_beta, w_qkv, w_proj, b_proj, HW)
    res = run_bass_kernel_spmd(nc, in_maps, list(range(NCORES)))
    return assemble_output(res.results, HW, Himg, Wimg)
